# revision 1
# baseline (speedup 1.0000x reference)
"""Trainium2 Bass kernel for nn_Decoder_3539053052044.

Structure (validated against the reference in numpy first):
- The reference decoder has a preserved bug: every layer consumes the ORIGINAL
  x0, so only the LAST layer's output survives. We compute layer L-1 only.
- Sequence-parallel: 8 cores x 256 tokens (core r -> batch r//4, chunk r%4).
  Each core computes the full last layer for its 256 tokens (K/V projections
  for its whole batch are computed locally), then one AllGather of y (bf16,
  0.5MB/rank), then a vocab-sharded projection (each core: all 2048 tokens x
  its 4000 vocab columns).
- Activations are feature-major [D on partitions, tokens free] so every linear
  layer uses the stored [D_in, D_out] weights directly as lhsT.
- Softmax is max-free (scores are O(1) for this model; exp(-1e9)=0 handles
  masking) and computed directly transposed, scoresT[k,q], so no transposes
  are needed; the per-(head,q) 1/sum is applied after the AV matmul via a
  K=1-matmul partition broadcast. The two heads of a feature pair share one
  2-bank scoresT psum tile (one matmul group per bank — sharing a bank
  between groups is a hardware fatal) so the mask-add and exp run as single
  [128,2,256] strided ops. V carries a ones column per head ([128, H*65])
  so the AV matmul's 65th output row is the softmax denominator for free.
- LayerNorm runs feature-major via ones-matmul partition reductions.
- bf16 matmul inputs, fp32 accumulation (measured rel err 2.9e-3 vs reference).
"""

import numpy as np
import ml_dtypes

import concourse.bass as bass
import concourse.bacc as bacc
import concourse.tile as tile
from concourse import mybir
from concourse.bass_utils import run_bass_kernel_spmd
from concourse.vector_clock import ScopedClock, VectorClock

BF16 = ml_dtypes.bfloat16
F32 = mybir.dt.float32
BF = mybir.dt.bfloat16
PSUM = bass.MemorySpace.PSUM

B, S, D, H, L, V, DF = 2, 1024, 1024, 16, 4, 32000, 4096
DH = D // H              # 64
NC = 8                   # cores
TOK = B * S // NC        # 256 tokens per core
VS = V // NC             # 4000 vocab cols per core
KT = S // 128            # 8 k tiles
FT = D // 128            # 8 feature tiles
HT = DF // 128           # 32 hidden tiles
VN = 8                   # vocab n-chunks
VC = VS // VN            # 500 cols per chunk
ADD = mybir.AluOpType.add
MULT = mybir.AluOpType.mult
IDENT = mybir.ActivationFunctionType.Identity

_PATCHED = False


def _patch_tile_drain():
    """This neuronxcc build rejects a Drain carrying >1 sem wait. Split the
    Tile tail drain into one Drain per busy proc, each with a single wait."""
    global _PATCHED
    if _PATCHED:
        return
    _PATCHED = True

    def _drain_and_barrier_split(self, tick_clock, wait_clock):
        gc = tick_clock.global_clock
        n = len(gc)
        for p in range(n):
            if gc[p] > 0:
                vc = VectorClock([gc[q] if q == p else 0 for q in range(n)])
                d = self.nc.sync.drain()
                wait_clock.add_sem_waits(d.ins, ScopedClock({None: vc}))
        self.nc.sync.drain()
        self.nc.all_engine_barrier()
        assert self.sems is not None
        popped = self.nc._tile_sem_poison_stack.pop()
        assert popped is self._sem_poison
        self.nc.clear_and_free_semaphores(list(self.sems.allocated().values()))
        self.nc.all_engine_barrier()

    tile.TileContext._drain_and_barrier = _drain_and_barrier_split


def positional_encoding(seq_len, d_model, n=10000.0):
    i = np.arange(seq_len, dtype=np.float32)[:, None]
    d = np.arange(d_model)
    denom = np.power(n, (2 * (d // 2)).astype(np.float32) / d_model)
    ang = i / denom
    return np.where(d % 2 == 0, np.sin(ang), np.cos(ang)).astype(np.float32)


BIAS_NAMES = ['sbk', 'sbq', 'sbo', 'cbk', 'cbq', 'cbo', 'fb2',
              'ln1_g', 'ln1_b', 'ln2_g', 'ln2_b', 'ln3_g', 'ln3_b']


def build_program(self_mask_adds: bool, cross_mask_adds: bool, zero_free_biases: bool = False,
                  stop_phase: int = 99):
    _patch_tile_drain()
    nc = bacc.Bacc()

    g = {}  # dram handles
    g['x0fm'] = nc.declare_dram_parameter("x0fm", [D, S], BF, isOutput=False)
    g['encfm'] = nc.declare_dram_parameter("encfm", [D, S], BF, isOutput=False)
    g['x0chunk'] = nc.declare_dram_parameter("x0chunk", [D, TOK], F32, isOutput=False)
    for w in ['sWq', 'sWk', 'sWv', 'sWo', 'cWq', 'cWk', 'cWv', 'cWo']:
        g[w] = nc.declare_dram_parameter(w, [D, D], BF, isOutput=False)
    g['fW1'] = nc.declare_dram_parameter("fW1", [D, DF], BF, isOutput=False)
    g['fW2'] = nc.declare_dram_parameter("fW2", [DF, D], BF, isOutput=False)
    g['Wout'] = nc.declare_dram_parameter("Wout", [D, VS], BF, isOutput=False)
    g['biases'] = nc.declare_dram_parameter("biases", [128, 8 * len(BIAS_NAMES)], F32, isOutput=False)
    g['fb1'] = nc.declare_dram_parameter("fb1", [128, HT], F32, isOutput=False)
    g['sbv_row'] = nc.declare_dram_parameter("sbv_row", [1, D], F32, isOutput=False)
    g['cbv_row'] = nc.declare_dram_parameter("cbv_row", [1, D], F32, isOutput=False)
    g['bout_row'] = nc.declare_dram_parameter("bout_row", [1, VS], F32, isOutput=False)
    g['maskT'] = nc.declare_dram_parameter("maskT", [S, 2 * TOK], F32, isOutput=False) if self_mask_adds else None
    g['maskTc'] = nc.declare_dram_parameter("maskTc", [S, 2 * TOK], F32, isOutput=False) if cross_mask_adds else None
    g['out'] = nc.declare_dram_parameter("out", [NC * TOK, VS], F32, isOutput=True)
    g['y_sh'] = nc.dram_tensor("y_sh", [D, TOK], BF)
    g['y_ag'] = nc.dram_tensor("y_ag", [NC, D, TOK], BF, addr_space="Shared")

    with tile.TileContext(nc) as tc:
        _emit(nc, tc, g, zero_free_biases, stop_phase)
    nc.compile()
    return nc


class _StopEmit(Exception):
    pass


def _emit(nc, tc, g, zero_free_biases, stop_phase=99):
    try:
        _emit_inner(nc, tc, g, zero_free_biases, stop_phase)
    except _StopEmit:
        pass


def _emit_inner(nc, tc, g, zero_free_biases, stop_phase):
    def phase_gate(p):
        if stop_phase < p:
            raise _StopEmit()
    from contextlib import ExitStack
    ctx = ExitStack()
    with ctx:
        # ---------- whole-kernel constants / small tensors ------------------
        const = ctx.enter_context(tc.tile_pool(name="const", bufs=1))
        ones_bf = const.tile([128, 1], BF, name="ones_bf", tag="c0")
        nc.gpsimd.memset(ones_bf[:], 1.0)
        ones_f32 = const.tile([128, 1], F32, name="ones_f32", tag="c1")
        nc.gpsimd.memset(ones_f32[:], 1.0)
        ones_row = const.tile([1, 128], F32, name="ones_row", tag="c2")
        nc.gpsimd.memset(ones_row[:], 1.0)
        bias_sb = const.tile([128, 8 * len(BIAS_NAMES)], F32, name="bias_sb", tag="c3")
        nc.sync.dma_start(bias_sb[:], g['biases'][:])
        fb1_sb = const.tile([128, HT], F32, name="fb1_sb", tag="c4")
        nc.sync.dma_start(fb1_sb[:], g['fb1'][:])
        def bias_col(name, f):
            i = BIAS_NAMES.index(name)
            return bias_sb[:, i * 8 + f:i * 8 + f + 1]

        # free-axis bias broadcast tiles [128, D] for sbv / cbv (skipped when
        # the host observed all-zero free-axis biases)
        free_bias = {'sbv': None, 'cbv': None}
        if not zero_free_biases:
            with tc.tile_pool(name="bbc_ps", bufs=1, space=PSUM) as bps, \
                 tc.tile_pool(name="bbc_row", bufs=2) as brow:
                for bi, bname in enumerate(['sbv', 'cbv']):
                    t = const.tile([128, D], F32, name=f"{bname}_b", tag=f"fb{bi}")
                    rsb = brow.tile([1, D], F32, tag="row")
                    nc.sync.dma_start(rsb[:], g[f'{bname}_row'][:])
                    for half in range(2):
                        ps = bps.tile([128, 512], F32, tag="bc")
                        nc.tensor.matmul(ps[:], ones_row[:],
                                         rsb[0:1, half * 512:(half + 1) * 512],
                                         start=True, stop=True)
                        nc.vector.tensor_copy(t[:, half * 512:(half + 1) * 512], ps[:])
                    free_bias[bname] = t

        def copy_out(ot, ps, idx):
            """psum->sbuf copy alternating DVE/ACT to balance engines"""
            if idx % 2 == 0:
                nc.vector.tensor_copy(ot, ps)
            else:
                nc.scalar.activation(ot, ps, IDENT)

        def load_w(w_name, pool, kt_n, cols, parts=1):
            """Load [kt_n*128, cols] weight as `parts` batched tiles.
            Returns accessor: wslice(k, c0, c1) -> [128, c1-c0] lhsT/rhs AP."""
            per = kt_n // parts
            tiles = []
            for pi in range(parts):
                wt = pool.tile([128, per, cols], BF, name=f"w_{w_name}_{pi}", tag=f"w{pi}")
                nc.sync.dma_start(
                    wt[:], g[w_name].rearrange("(a p) d -> p a d", p=128)[:, pi * per:(pi + 1) * per, :])
                tiles.append(wt)

            def wslice(k, c0, c1):
                return tiles[k // per][:, k % per, c0:c1]
            return wslice

        # LN outputs (outer lifetime)
        a1pool = ctx.enter_context(tc.tile_pool(name="a1", bufs=FT))
        a2pool = ctx.enter_context(tc.tile_pool(name="a2", bufs=FT))
        ypool = ctx.enter_context(tc.tile_pool(name="y", bufs=FT))

        # ---------- helpers -------------------------------------------------
        def proj_fm(w_name, act_tiles, n_tok, bias_name, out_pool, scale=None):
            """feature-major out tiles [FT x [128, n_tok]] bf16 = W.T @ act + b"""
            outs = []
            nsub = (n_tok + 511) // 512
            with tc.tile_pool(name=f"w_{w_name}", bufs=1) as wp, \
                 tc.tile_pool(name=f"ps_{w_name}", bufs=4, space=PSUM) as pp:
                w = load_w(w_name, wp, FT, D, parts=2)
                for m in range(FT):
                    ot = out_pool.tile([128, n_tok], BF, name=f"o_{w_name}_{m}",
                                       tag=f"o_{w_name}_{m}", bufs=1)
                    for ns in range(nsub):
                        c0, c1 = ns * 512, min((ns + 1) * 512, n_tok)
                        ps = pp.tile([128, c1 - c0], F32, tag="ps")
                        for k in range(FT):
                            nc.tensor.matmul(ps[:], w(k, m * 128, (m + 1) * 128),
                                             act_tiles[k][:, c0:c1],
                                             start=(k == 0), stop=(k == FT - 1))
                        if scale is not None:
                            nc.vector.tensor_scalar(ot[:, c0:c1], ps[:], scale,
                                                    bias_col(bias_name, m), MULT, ADD)
                        else:
                            nc.scalar.activation(ot[:, c0:c1], ps[:], IDENT,
                                                 bias=bias_col(bias_name, m))
                    outs.append(ot)
            return outs

        def proj_tm(w_name, act_tiles, bias_bcast, out_pool):
            """token-major V tiles [KT x [128, H*65]] bf16 = act.T @ W + b, with
            a ones column appended after each head's 64 dims so the AV matmul's
            65th output row is the softmax denominator for free."""
            outs = []
            with tc.tile_pool(name=f"w_{w_name}", bufs=1) as wp, \
                 tc.tile_pool(name=f"ps_{w_name}", bufs=4, space=PSUM) as pp:
                w = load_w(w_name, wp, FT, D, parts=2)
                for m in range(KT):
                    ot = out_pool.tile([128, H * 65], BF, name=f"o_{w_name}_{m}",
                                       tag=f"o_{w_name}_{m}", bufs=1)
                    ones_cols = ot.rearrange("p (h c) -> p h c", c=65)[:, :, 64:65]
                    nc.gpsimd.memset(ones_cols, 1.0)
                    for ns in range(2):
                        c0, c1 = ns * 512, (ns + 1) * 512
                        ps = pp.tile([128, 512], F32, tag="ps")
                        for k in range(FT):
                            nc.tensor.matmul(ps[:], act_tiles[k][:, m * 128:(m + 1) * 128],
                                             w(k, c0, c1),
                                             start=(k == 0), stop=(k == FT - 1))
                        dst = ot[:, ns * 8 * 65:(ns * 8 + 8) * 65].rearrange(
                            "p (h c) -> p h c", c=65)[:, :, 0:64]
                        psv = ps.rearrange("p (h c) -> p h c", c=64)
                        if bias_bcast is None:
                            copy_out(dst, psv, m * 2 + ns)
                        else:
                            bbv = bias_bcast[:, c0:c1].rearrange("p (h c) -> p h c", c=64)
                            nc.vector.scalar_tensor_tensor(dst, psv, 1.0, bbv, MULT, ADD)
                    outs.append(ot)
            return outs

        def attention(q_pairs, k_tiles, v_tiles, mask_tiles, tag, out_pool):
            """q_pairs feature-major [FT x [128,TOK]] bf16; k_tiles [FT x [128,S]];
            v_tiles token-major [KT x [128,D]]; mask_tiles doubled [KT x [128,2*TOK]].
            Both heads of a feature pair are processed together: one [128,2*TOK]
            scoresT psum per k-tile -> one mask add -> one exp; AV packs the two
            heads into one [128,TOK] psum via column groups."""
            outs = []
            with tc.tile_pool(name=f"exp_{tag}", bufs=4) as epool, \
                 tc.tile_pool(name=f"asm_{tag}", bufs=4) as spool, \
                 tc.tile_pool(name=f"sT_{tag}", bufs=3, space=PSUM) as sps, \
                 tc.tile_pool(name=f"av_{tag}", bufs=2, space=PSUM) as avs:
                for hp in range(FT):
                    at = out_pool.tile([128, TOK], BF, name=f"at_{tag}_{hp}",
                                       tag=f"at_{hp}", bufs=1)
                    av0 = avs.tile([65, TOK], F32, tag="av")
                    av1 = avs.tile([65, TOK], F32, tag="av")
                    for kt in range(KT):
                        # two heads' scoresT in one 2-bank psum tile (one matmul
                        # group per bank); mask-add + exp fused via strided APs
                        sTp = sps.tile([128, 4 * TOK], F32, tag="sT")
                        for hh in range(2):
                            po = hh * 64
                            nc.tensor.matmul(
                                sTp[:, hh * 2 * TOK:hh * 2 * TOK + TOK],
                                k_tiles[hp][po:po + 64, kt * 128:(kt + 1) * 128],
                                q_pairs[hp][po:po + 64, :], start=True, stop=True)
                        sview = sTp.rearrange("p (b c) -> p b c", c=2 * TOK)[:, :, 0:TOK]
                        ex = epool.tile([128, 2 * TOK], BF, tag="exp")
                        exv = ex.rearrange("p (b c) -> p b c", c=TOK)
                        if mask_tiles is not None:
                            # stage the add in SBUF so the scoresT psum slot is
                            # released after the DVE add, not after the ACT exp
                            st_sb = epool.tile([128, 2 * TOK], F32, tag="st_sb")
                            sbv = st_sb.rearrange("p (b c) -> p b c", c=TOK)
                            mview = mask_tiles[kt].rearrange("p (b c) -> p b c", c=TOK)
                            nc.vector.tensor_add(sbv, sview, mview)
                            nc.scalar.activation(exv, sbv, mybir.ActivationFunctionType.Exp)
                        else:
                            nc.scalar.activation(exv, sview, mybir.ActivationFunctionType.Exp)
                        for hh, av in ((0, av0), (1, av1)):
                            h = 2 * hp + hh
                            nc.tensor.matmul(av[:],
                                             v_tiles[kt][:, h * 65:(h + 1) * 65],
                                             ex[:, hh * TOK:(hh + 1) * TOK],
                                             start=(kt == 0), stop=(kt == KT - 1))
                    for hh, av in ((0, av0), (1, av1)):
                        rec = spool.tile([1, TOK], F32, tag="recip")
                        nc.vector.reciprocal(rec[:], av[64:65, :])
                        rb = sps.tile([64, TOK], F32, tag="sT")
                        nc.tensor.matmul(rb[:], ones_row[0:1, 0:64], rec[:],
                                         start=True, stop=True)
                        rb_sb = spool.tile([64, TOK], F32, tag="rb_sb")
                        nc.vector.tensor_copy(rb_sb[:], rb[:])
                        nc.vector.tensor_mul(at[hh * 64:hh * 64 + 64, :], av[0:64, :], rb_sb[:])
                    outs.append(at)
            return outs

        def o_proj_residual(w_name, attn_tiles, bo_name, resid_tiles, rpool):
            outs = []
            with tc.tile_pool(name=f"w_{w_name}", bufs=1) as wp, \
                 tc.tile_pool(name=f"ps_{w_name}", bufs=4, space=PSUM) as pp:
                w = load_w(w_name, wp, FT, D, parts=2)
                for m in range(FT):
                    ps = pp.tile([128, TOK], F32, tag="ps")
                    for k in range(FT):
                        nc.tensor.matmul(ps[:], w(k, m * 128, (m + 1) * 128),
                                         attn_tiles[k][:], start=(k == 0), stop=(k == FT - 1))
                    rt = rpool.tile([128, TOK], F32, name=f"r_{w_name}_{m}", tag=f"r{m}")
                    nc.vector.scalar_tensor_tensor(rt[:], ps[:], bias_col(bo_name, m),
                                                   resid_tiles[m][:], ADD, ADD)
                    outs.append(rt)
            return outs

        def layer_norm(r_tiles, g_name, b_name, out_dtype, out_pool, want_bf16):
            with tc.tile_pool(name=f"lnp_{g_name}", bufs=1, space=PSUM) as lnps, \
                 tc.tile_pool(name=f"lnb_{g_name}", bufs=1, space=PSUM) as lnbc, \
                 tc.tile_pool(name=f"lns_{g_name}", bufs=2) as lnsm, \
                 tc.tile_pool(name=f"lnq_{g_name}", bufs=3) as sqp:
                s1 = lnps.tile([1, TOK], F32, tag="s1")
                s2 = lnps.tile([1, TOK], F32, tag="s2")
                for k in range(FT):
                    nc.tensor.matmul(s1[:], ones_f32[:], r_tiles[k][:],
                                     start=(k == 0), stop=(k == FT - 1))
                for k in range(FT):
                    sq = sqp.tile([128, TOK], F32, tag="sq")
                    nc.vector.tensor_mul(sq[:], r_tiles[k][:], r_tiles[k][:])
                    nc.tensor.matmul(s2[:], ones_f32[:], sq[:],
                                     start=(k == 0), stop=(k == FT - 1))
                mean = lnsm.tile([1, TOK], F32, tag="mean")
                nc.vector.tensor_scalar_mul(mean[:], s1[:], 1.0 / D)
                var = lnsm.tile([1, TOK], F32, tag="var")
                # var = s2/D - mean^2  ==  (s2 * 1/D) + (-mean*mean)
                nc.vector.scalar_tensor_tensor(var[:], mean[:], -1.0, mean[:], MULT, MULT)
                nc.vector.scalar_tensor_tensor(var[:], s2[:], 1.0 / D, var[:], MULT, ADD)
                nc.vector.tensor_scalar_add(var[:], var[:], 1e-5)
                std = lnsm.tile([1, TOK], F32, tag="std")
                nc.scalar.activation(std[:], var[:], mybir.ActivationFunctionType.Sqrt)
                rstd = lnsm.tile([1, TOK], F32, tag="rstd")
                nc.vector.reciprocal(rstd[:], std[:])
                mean_b = lnbc.tile([128, TOK], F32, tag="meanb")
                nc.tensor.matmul(mean_b[:], ones_row[:], mean[:], start=True, stop=True)
                rstd_b = lnbc.tile([128, TOK], F32, tag="rstdb")
                nc.tensor.matmul(rstd_b[:], ones_row[:], rstd[:], start=True, stop=True)
                outs, outs_bf = [], []
                for k in range(FT):
                    xn = sqp.tile([128, TOK], F32, tag="xn")
                    nc.vector.tensor_sub(xn[:], r_tiles[k][:], mean_b[:])
                    nc.vector.tensor_mul(xn[:], xn[:], rstd_b[:])
                    ot = out_pool.tile([128, TOK], out_dtype, name=f"ln_{g_name}_{k}",
                                       tag=f"ln_{k}", bufs=1)
                    nc.vector.tensor_scalar(ot[:], xn[:], bias_col(g_name, k),
                                            bias_col(b_name, k), MULT, ADD)
                    outs.append(ot)
                    if want_bf16:
                        ob = out_pool.tile([128, TOK], BF, name=f"lnb_{g_name}_{k}",
                                           tag=f"lnbf_{k}", bufs=1)
                        nc.scalar.activation(ob[:], ot[:], IDENT)
                        outs_bf.append(ob)
            return outs, outs_bf

        # ================= phase 1-3: attention blocks ======================
        with tc.tile_pool(name="kvc", bufs=1) as kvc_pool:
            with tc.tile_pool(name="kvs", bufs=1) as kvs_pool, \
                 tc.tile_pool(name="x0c", bufs=1) as x0c_pool, \
                 tc.tile_pool(name="mask", bufs=1) as mask_pool, \
                 tc.tile_pool(name="r1p", bufs=1) as r1_pool:
                x0c_b = x0c_pool.tile([128, FT, TOK], F32, name="x0c_b", tag="x0c")
                nc.sync.dma_start(x0c_b[:], g['x0chunk'].rearrange("(a p) t -> p a t", p=128)[:])
                x0c_t = [x0c_b[:, k, :] for k in range(FT)]
                maskT_t = None
                if g['maskT'] is not None:
                    mt_b = mask_pool.tile([128, KT, 2 * TOK], F32, name="mt_b", tag="mt")
                    nc.sync.dma_start(mt_b[:], g['maskT'].rearrange("(a p) t -> p a t", p=128)[:])
                    maskT_t = [mt_b[:, k, :] for k in range(KT)]

                # phase 1: K/V/Q projections (x0 first, release, then enc)
                with tc.tile_pool(name="x0cb", bufs=1) as xcb_pool:
                    x0cb = []
                    for k in range(FT):
                        t = xcb_pool.tile([128, TOK], BF, name=f"x0cb_{k}", tag=f"b{k}")
                        nc.vector.tensor_copy(t[:], x0c_t[k][:])
                        x0cb.append(t)
                    q_self = proj_fm('sWq', x0cb, TOK, 'sbq', kvs_pool, scale=0.125)
                with tc.tile_pool(name="acts_x0", bufs=1) as actp:
                    x0_b = actp.tile([128, FT, S], BF, name="x0_b", tag="x0")
                    x0r = g['x0fm'].rearrange("(a p) t -> p a t", p=128)
                    nc.sync.dma_start(x0_b[:, 0:4, :], x0r[:, 0:4, :])
                    nc.sync.dma_start(x0_b[:, 4:8, :], x0r[:, 4:8, :])
                    x0_t = [x0_b[:, k, :] for k in range(FT)]
                    k_self = proj_fm('sWk', x0_t, S, 'sbk', kvs_pool)
                    v_self = proj_tm('sWv', x0_t, free_bias['sbv'], kvs_pool)

                phase_gate(1)
                # phase 2: self attention + O-proj + LN1
                with tc.tile_pool(name="at_s", bufs=1) as at_pool_s:
                    attn1 = attention(q_self, k_self, v_self, maskT_t, "s", at_pool_s)
                    r1 = o_proj_residual('sWo', attn1, 'sbo', x0c_t, r1_pool)
                a1, a1b = layer_norm(r1, 'ln1_g', 'ln1_b', F32, a1pool, True)

                with tc.tile_pool(name="acts_enc", bufs=1) as actp:
                    enc_b = actp.tile([128, FT, S], BF, name="enc_b", tag="enc")
                    encr = g['encfm'].rearrange("(a p) t -> p a t", p=128)
                    nc.sync.dma_start(enc_b[:, 0:4, :], encr[:, 0:4, :])
                    nc.sync.dma_start(enc_b[:, 4:8, :], encr[:, 4:8, :])
                    enc_t = [enc_b[:, k, :] for k in range(FT)]
                    k_cross = proj_fm('cWk', enc_t, S, 'cbk', kvc_pool)
                    v_cross = proj_tm('cWv', enc_t, free_bias['cbv'], kvc_pool)

            phase_gate(2)
            # phase 3: cross attention + O-proj + LN2
            with tc.tile_pool(name="qc", bufs=1) as qc_pool, \
                 tc.tile_pool(name="maskc", bufs=1) as maskc_pool, \
                 tc.tile_pool(name="r2p", bufs=1) as r2_pool:
                maskTc_t = None
                if g['maskTc'] is not None:
                    mtc_b = maskc_pool.tile([128, KT, 2 * TOK], F32, name="mtc_b", tag="mtc")
                    nc.sync.dma_start(mtc_b[:], g['maskTc'].rearrange("(a p) t -> p a t", p=128)[:])
                    maskTc_t = [mtc_b[:, k, :] for k in range(KT)]
                q_cross = proj_fm('cWq', a1b, TOK, 'cbq', qc_pool, scale=0.125)
                with tc.tile_pool(name="at_c", bufs=1) as at_pool_c:
                    attn2 = attention(q_cross, k_cross, v_cross, maskTc_t, "c", at_pool_c)
                    r2 = o_proj_residual('cWo', attn2, 'cbo', a1, r2_pool)
                a2, a2b = layer_norm(r2, 'ln2_g', 'ln2_b', F32, a2pool, True)

        phase_gate(3)
        # ================= phase 4: FFN + LN3 ===============================
        with tc.tile_pool(name="hid", bufs=1) as hpool:
            h_tiles = []
            with tc.tile_pool(name="w_fW1", bufs=1) as wp1, \
                 tc.tile_pool(name="ps_f1", bufs=3, space=PSUM) as pp1:
                w1 = load_w('fW1', wp1, FT, DF, parts=4)
                for m in range(HT):
                    ps = pp1.tile([128, TOK], F32, tag="ps")
                    for k in range(FT):
                        nc.tensor.matmul(ps[:], w1(k, m * 128, (m + 1) * 128),
                                         a2b[k][:], start=(k == 0), stop=(k == FT - 1))
                    ht = hpool.tile([128, TOK], BF, name=f"h_{m}", tag=f"h_{m}")
                    nc.scalar.activation(ht[:], ps[:], mybir.ActivationFunctionType.Relu,
                                         bias=fb1_sb[:, m:m + 1])
                    h_tiles.append(ht)
            r3 = []
            with tc.tile_pool(name="w_fW2", bufs=1) as wp2, \
                 tc.tile_pool(name="r3p", bufs=1) as r3_pool, \
                 tc.tile_pool(name="ps_f2", bufs=3, space=PSUM) as pp2:
                w2 = load_w('fW2', wp2, HT, D, parts=4)
                for m in range(FT):
                    ps = pp2.tile([128, TOK], F32, tag="ps")
                    for kh in range(HT):
                        nc.tensor.matmul(ps[:], w2(kh, m * 128, (m + 1) * 128),
                                         h_tiles[kh][:], start=(kh == 0), stop=(kh == HT - 1))
                    rt = r3_pool.tile([128, TOK], F32, name=f"r_ffn_{m}", tag=f"r{m}")
                    nc.vector.scalar_tensor_tensor(rt[:], ps[:], bias_col('fb2', m),
                                                   a2[m][:], ADD, ADD)
                    r3.append(rt)
                y, _ = layer_norm(r3, 'ln3_g', 'ln3_b', BF, ypool, False)

        phase_gate(4)
        # ================= phase 5: AllGather of y ==========================
        for k in range(FT):
            nc.sync.dma_start(g['y_sh'][k * 128:(k + 1) * 128, :], y[k][:])
        nc.gpsimd.collective_compute(
            "AllGather", mybir.AluOpType.bypass,
            replica_groups=[list(range(NC))],
            ins=[g['y_sh'][:]], outs=[g['y_ag'][:]])

        phase_gate(5)
        # ================= phase 6: vocab projection ========================
        with tc.tile_pool(name="yg", bufs=1) as ygp, \
             tc.tile_pool(name="wout", bufs=1) as woutp, \
             tc.tile_pool(name="vout", bufs=2) as vos, \
             tc.tile_pool(name="vps", bufs=4, space=PSUM) as vps, \
             tc.tile_pool(name="bps", bufs=1, space=PSUM) as bps:
            yg = []
            for r in range(NC):
                t = ygp.tile([128, FT, TOK], BF, name=f"yg_{r}", tag=f"yg_{r}")
                nc.sync.dma_start(t[:], g['y_ag'][r].rearrange("(a p) t -> p a t", p=128)[:])
                yg.append(t)
            wv = load_w('Wout', woutp, FT, VS, parts=4)
            bout_b = None
            if not zero_free_biases:
                brow = woutp.tile([1, VS], F32, name="brow", tag="brow")
                nc.sync.dma_start(brow[:], g['bout_row'][:])
                bout_b = woutp.tile([128, VS], F32, name="boutb", tag="boutb")
                for n in range(VN):
                    bp = bps.tile([128, VC], F32, tag="bb")
                    nc.tensor.matmul(bp[:], ones_row[:], brow[0:1, n * VC:(n + 1) * VC],
                                     start=True, stop=True)
                    nc.vector.tensor_copy(bout_b[:, n * VC:(n + 1) * VC], bp[:])
            for m in range(NC * TOK // 128):
                r, half = m // 2, m % 2
                ot = vos.tile([128, VS], F32, tag="vo")
                for n in range(VN):
                    ps = vps.tile([128, VC], F32, tag="ps")
                    for k in range(FT):
                        nc.tensor.matmul(ps[:], yg[r][:, k, half * 128:(half + 1) * 128],
                                         wv(k, n * VC, (n + 1) * VC),
                                         start=(k == 0), stop=(k == FT - 1))
                    if bout_b is None:
                        copy_out(ot[:, n * VC:(n + 1) * VC], ps[:], n)
                    else:
                        nc.vector.scalar_tensor_tensor(ot[:, n * VC:(n + 1) * VC], ps[:],
                                                       1.0, bout_b[:, n * VC:(n + 1) * VC],
                                                       MULT, ADD)
                nc.sync.dma_start(g['out'][m * 128:(m + 1) * 128, :], ot[:])


def host_prep(inputs):
    x0 = np.asarray(inputs['dec_input'], np.float32) + positional_encoding(S, D)[None]
    enc = np.asarray(inputs['enc_input'], np.float32)
    mask_self = np.asarray(inputs['masked_attention_mask'], np.float32)[0, 0]
    mask_cross = np.asarray(inputs['cross_attention_mask'], np.float32)[0, 0]
    self_adds = bool(np.any(mask_self != 0.0))
    cross_adds = bool(np.any(mask_cross != 0.0))
    li = L - 1
    Wl = {}
    for p in ['sWq', 'sWk', 'sWv', 'sWo', 'cWq', 'cWk', 'cWv', 'cWo', 'fW1', 'fW2']:
        Wl[p] = np.ascontiguousarray(np.asarray(inputs[p], np.float32)[li]).astype(BF16)
    bv = {}
    for p in ['sbq', 'sbk', 'sbv', 'sbo', 'cbq', 'cbk', 'cbv', 'cbo',
              'ln1_g', 'ln1_b', 'ln2_g', 'ln2_b', 'ln3_g', 'ln3_b', 'fb1', 'fb2']:
        bv[p] = np.asarray(inputs[p], np.float32)[li]
    Wout = np.asarray(inputs['Wout'], np.float32)
    bout = np.asarray(inputs['bout'], np.float32)

    def pp(v):  # [1024] -> [128, 8] partition-major
        return np.ascontiguousarray(v.reshape(-1, 128).T)

    bias_cols = []
    for name in BIAS_NAMES:
        src = {'sbq': bv['sbq'] * 0.125, 'cbq': bv['cbq'] * 0.125}.get(name, bv.get(name))
        bias_cols.append(pp(src))
    biases_pp = np.ascontiguousarray(np.concatenate(bias_cols, axis=1), np.float32)
    fb1_pp = np.ascontiguousarray(bv['fb1'].reshape(HT, 128).T, np.float32)

    in_maps = []
    for core in range(NC):
        b, c = core // 4, core % 4
        q0 = c * TOK
        m = {
            'x0fm': np.ascontiguousarray(x0[b].T).astype(BF16),
            'encfm': np.ascontiguousarray(enc[b].T).astype(BF16),
            'x0chunk': np.ascontiguousarray(x0[b, q0:q0 + TOK].T, np.float32),
            'biases': biases_pp, 'fb1': fb1_pp,
            'sbv_row': np.ascontiguousarray(bv['sbv'][None, :], np.float32),
            'cbv_row': np.ascontiguousarray(bv['cbv'][None, :], np.float32),
            'bout_row': np.ascontiguousarray(bout[None, core * VS:(core + 1) * VS], np.float32),
            'Wout': np.ascontiguousarray(Wout[:, core * VS:(core + 1) * VS]).astype(BF16),
        }
        m.update(Wl)
        if self_adds:
            mt = mask_self[q0:q0 + TOK, :].T
            m['maskT'] = np.ascontiguousarray(np.concatenate([mt, mt], axis=1), np.float32)
        if cross_adds:
            mt = mask_cross[q0:q0 + TOK, :].T
            m['maskTc'] = np.ascontiguousarray(np.concatenate([mt, mt], axis=1), np.float32)
        in_maps.append(m)
    zero_free = not (np.any(bv['sbv']) or np.any(bv['cbv']) or np.any(bout))
    return in_maps, self_adds, cross_adds, zero_free


_CACHE = {}


def _get_program(self_adds, cross_adds, zero_free):
    key = (self_adds, cross_adds, zero_free)
    if key not in _CACHE:
        _CACHE[key] = build_program(self_adds, cross_adds, zero_free)
    return _CACHE[key]


def kernel(**inputs):
    in_maps, self_adds, cross_adds, zero_free = host_prep(inputs)
    nc = _get_program(self_adds, cross_adds, zero_free)
    res = run_bass_kernel_spmd(nc, in_maps, core_ids=list(range(NC)))
    shards = [res.results[r]["out"] for r in range(NC)]
    full = np.concatenate(shards, axis=1)          # [2048, V]
    return np.ascontiguousarray(full.reshape(B, S, V), np.float32)



# revision 12
# speedup vs baseline: 1.2618x; 1.2618x over previous
"""Trainium2 Bass kernel for nn_Decoder_3539053052044.

Structure (v2 — no collectives):
- The reference decoder has a preserved bug: every layer consumes the ORIGINAL
  x0, so only the LAST layer's output survives. We compute layer L-1 only.
- Sequence-parallel: 8 cores x 256 tokens (core r -> batch r//4, chunk r%4).
  Each core computes the full last layer for its 256 tokens (K/V projections
  for its whole batch are computed locally), then projects its OWN 256 tokens
  against the FULL vocab (full Wout streamed from HBM in 500-col chunks
  through an 8-deep SBUF ring) — zero cross-core communication.
- Activations are feature-major [D on partitions, tokens free] so every linear
  layer uses the stored [D_in, D_out] weights directly as lhsT.
- Softmax is max-free (scores are O(1) for this model; exp(-1e9)=0 handles
  masking) and computed directly transposed, scoresT[k,q]. The additive mask
  is folded into the scores PSUM accumulation group via an identity-weight
  matmul (PE), so exp reads PSUM directly — no DVE mask-add, no SBUF staging.
  V carries a ones column per head ([128, H*65]) so the AV matmul's 65th
  output row is the softmax denominator for free.
- LayerNorm / broadcast matmuls run as float32r (1 cycle/row at free>=256 in
  the PE, vs 4 for plain fp32).
- FFN weights are streamed in 4 hidden-chunks (2MB each, double buffered) with
  the fW2 contraction accumulated into SBUF, freeing SBUF for the Wout ring.
- Output is written bf16 ([256, 32000] per core) and converted to fp32 on the
  host; vocab bias (all-zero here) would be added host-side.
- bf16 matmul inputs, fp32 accumulation.
"""

import numpy as np
import ml_dtypes

import concourse.bass as bass
import concourse.bacc as bacc
import concourse.tile as tile
from concourse import mybir
from concourse.bass_utils import run_bass_kernel_spmd
from concourse.vector_clock import ScopedClock, VectorClock

BF16 = ml_dtypes.bfloat16
F32 = mybir.dt.float32
F32R = mybir.dt.float32r
BF = mybir.dt.bfloat16
PSUM = bass.MemorySpace.PSUM

B, S, D, H, L, V, DF = 2, 1024, 1024, 16, 4, 32000, 4096
DH = D // H              # 64
NC = 8                   # cores
TOK = B * S // NC        # 256 tokens per core
KT = S // 128            # 8 k tiles
FT = D // 128            # 8 feature tiles
HT = DF // 128           # 32 hidden tiles
NCH = 4                  # FFN hidden chunks
CHT = HT // NCH          # 8 hidden tiles per chunk
VC = 500                 # vocab cols per chunk
VN = V // VC             # 64 vocab chunks
RING = 12                # Wout ring depth
ADD = mybir.AluOpType.add
MULT = mybir.AluOpType.mult
IDENT = mybir.ActivationFunctionType.Identity
ADD_OP = mybir.AluOpType.add

_PATCHED = False


def _patch_tile_drain():
    """This neuronxcc build rejects a Drain carrying >1 sem wait. Split the
    Tile tail drain into one Drain per busy proc, each with a single wait."""
    global _PATCHED
    if _PATCHED:
        return
    _PATCHED = True

    def _drain_and_barrier_split(self, tick_clock, wait_clock):
        gc = tick_clock.global_clock
        n = len(gc)
        for p in range(n):
            if gc[p] > 0:
                vc = VectorClock([gc[q] if q == p else 0 for q in range(n)])
                d = self.nc.sync.drain()
                wait_clock.add_sem_waits(d.ins, ScopedClock({None: vc}))
        self.nc.sync.drain()
        self.nc.all_engine_barrier()
        assert self.sems is not None
        popped = self.nc._tile_sem_poison_stack.pop()
        assert popped is self._sem_poison
        self.nc.clear_and_free_semaphores(list(self.sems.allocated().values()))
        self.nc.all_engine_barrier()

    tile.TileContext._drain_and_barrier = _drain_and_barrier_split


def positional_encoding(seq_len, d_model, n=10000.0):
    i = np.arange(seq_len, dtype=np.float32)[:, None]
    d = np.arange(d_model)
    denom = np.power(n, (2 * (d // 2)).astype(np.float32) / d_model)
    ang = i / denom
    return np.where(d % 2 == 0, np.sin(ang), np.cos(ang)).astype(np.float32)


BIAS_NAMES = ['sbk', 'sbq', 'sbo', 'cbk', 'cbq', 'cbo', 'fb2',
              'ln1_g', 'ln1_b', 'ln2_g', 'ln2_b', 'ln3_g', 'ln3_b']


def build_program(self_mask_adds: bool, cross_mask_adds: bool, zero_free_biases: bool = False):
    _patch_tile_drain()
    nc = bacc.Bacc()

    g = {}  # dram handles
    g['x0fm'] = nc.declare_dram_parameter("x0fm", [D, S], BF, isOutput=False)
    g['encfm'] = nc.declare_dram_parameter("encfm", [D, S], BF, isOutput=False)
    g['x0chunk'] = nc.declare_dram_parameter("x0chunk", [D, TOK], F32, isOutput=False)
    g['x0cb'] = nc.declare_dram_parameter("x0cb", [D, TOK], BF, isOutput=False)
    for w in ['sWq', 'sWk', 'sWv', 'sWo', 'cWq', 'cWk', 'cWv', 'cWo']:
        g[w] = nc.declare_dram_parameter(w, [D, D], BF, isOutput=False)
    g['fW1'] = nc.declare_dram_parameter("fW1", [D, DF], BF, isOutput=False)
    g['fW2'] = nc.declare_dram_parameter("fW2", [DF, D], BF, isOutput=False)
    g['Wout'] = nc.declare_dram_parameter("Wout", [D, V], BF, isOutput=False)
    g['biases'] = nc.declare_dram_parameter("biases", [128, 8 * len(BIAS_NAMES)], F32, isOutput=False)
    g['fb1'] = nc.declare_dram_parameter("fb1", [128, HT], F32, isOutput=False)
    g['ident'] = nc.declare_dram_parameter("ident", [128, 128], BF, isOutput=False)
    g['sbv_row'] = nc.declare_dram_parameter("sbv_row", [1, D], F32, isOutput=False)
    g['cbv_row'] = nc.declare_dram_parameter("cbv_row", [1, D], F32, isOutput=False)
    g['maskT'] = nc.declare_dram_parameter("maskT", [S, 2 * TOK], BF, isOutput=False) if self_mask_adds else None
    g['maskTc'] = nc.declare_dram_parameter("maskTc", [S, 2 * TOK], BF, isOutput=False) if cross_mask_adds else None
    g['out'] = nc.declare_dram_parameter("out", [TOK, V], BF, isOutput=True)

    with tile.TileContext(nc) as tc:
        _emit(nc, tc, g, zero_free_biases)
    nc.compile()
    return nc


def _emit(nc, tc, g, zero_free_biases):
    from contextlib import ExitStack
    ctx = ExitStack()
    with ctx:
        # ---------- whole-kernel constants / small tensors ------------------
        const = ctx.enter_context(tc.tile_pool(name="const", bufs=1))
        ones_bf = const.tile([128, 1], BF, name="ones_bf", tag="c0")
        nc.gpsimd.memset(ones_bf[:], 1.0)
        ones_f32 = const.tile([128, 1], F32, name="ones_f32", tag="c1")
        nc.gpsimd.memset(ones_f32[:], 1.0)
        ones_row = const.tile([1, 128], F32, name="ones_row", tag="c2")
        nc.gpsimd.memset(ones_row[:], 1.0)
        ones_row_bf = const.tile([1, 128], BF, name="ones_row_bf", tag="c6")
        nc.gpsimd.memset(ones_row_bf[:], 1.0)
        bias_sb = const.tile([128, 8 * len(BIAS_NAMES)], F32, name="bias_sb", tag="c3")
        nc.sync.dma_start(bias_sb[:], g['biases'][:])
        fb1_sb = const.tile([128, HT], F32, name="fb1_sb", tag="c4")
        nc.sync.dma_start(fb1_sb[:], g['fb1'][:])
        ident_sb = const.tile([128, 128], BF, name="ident_sb", tag="c5")
        nc.sync.dma_start(ident_sb[:], g['ident'][:])

        def bias_col(name, f):
            i = BIAS_NAMES.index(name)
            return bias_sb[:, i * 8 + f:i * 8 + f + 1]

        # free-axis bias broadcast tiles [128, D] for sbv / cbv (skipped when
        # the host observed all-zero free-axis biases)
        free_bias = {'sbv': None, 'cbv': None}
        if not zero_free_biases:
            with tc.tile_pool(name="bbc_ps", bufs=1, space=PSUM) as bps, \
                 tc.tile_pool(name="bbc_row", bufs=2) as brow:
                for bi, bname in enumerate(['sbv', 'cbv']):
                    t = const.tile([128, D], F32, name=f"{bname}_b", tag=f"fb{bi}")
                    rsb = brow.tile([1, D], F32, tag="row")
                    nc.sync.dma_start(rsb[:], g[f'{bname}_row'][:])
                    for half in range(2):
                        ps = bps.tile([128, 512], F32, tag="bc")
                        nc.tensor.matmul(ps[:], ones_row[:],
                                         rsb[0:1, half * 512:(half + 1) * 512],
                                         start=True, stop=True)
                        nc.vector.tensor_copy(t[:, half * 512:(half + 1) * 512], ps[:])
                    free_bias[bname] = t

        # Wout ring: reserved at the bottom of the SBUF stack for the whole
        # program so the prefetch DMAs never anti-depend on phase pools.
        wo_pool = ctx.enter_context(tc.tile_pool(name="wo_ring", bufs=RING))
        wo_tiles = {}

        def emit_wo_load(n):
            t = wo_pool.tile([128, FT, VC], BF, name=f"wo_{n}", tag="wo")
            nc.sync.dma_start(
                t[:], g['Wout'].rearrange("(a p) v -> p a v", p=128)[:, :, n * VC:(n + 1) * VC])
            wo_tiles[n] = t

        def copy_out(ot, ps, idx):
            """psum->sbuf copy alternating DVE/ACT to balance engines"""
            if idx % 2 == 0:
                nc.vector.tensor_copy(ot, ps)
            else:
                nc.scalar.activation(ot, ps, IDENT)

        def load_w(w_name, pool, kt_n, cols, parts=1):
            """Load [kt_n*128, cols] weight as `parts` batched tiles.
            Returns accessor: wslice(k, c0, c1) -> [128, c1-c0] lhsT/rhs AP."""
            per = kt_n // parts
            tiles = []
            for pi in range(parts):
                wt = pool.tile([128, per, cols], BF, name=f"w_{w_name}_{pi}", tag=f"w{pi}")
                nc.sync.dma_start(
                    wt[:], g[w_name].rearrange("(a p) d -> p a d", p=128)[:, pi * per:(pi + 1) * per, :])
                tiles.append(wt)

            def wslice(k, c0, c1):
                return tiles[k // per][:, k % per, c0:c1]
            return wslice

        # LN outputs: a1 lives through phase 3; a2 through FFN; y to the end.
        ypool = ctx.enter_context(tc.tile_pool(name="y", bufs=FT))

        # ---------- helpers -------------------------------------------------
        def proj_fm(w_name, act_tiles, n_tok, bias_name, out_pool, scale=None,
                    tag_prefix=None):
            """feature-major out tiles [FT x [128, n_tok]] bf16 = W.T @ act + b"""
            outs = []
            tp = tag_prefix or f"o_{w_name}"
            nsub = (n_tok + 511) // 512
            with tc.tile_pool(name=f"w_{w_name}", bufs=1) as wp, \
                 tc.tile_pool(name=f"ps_{w_name}", bufs=4, space=PSUM) as pp:
                w = load_w(w_name, wp, FT, D, parts=2)
                for m in range(FT):
                    ot = out_pool.tile([128, n_tok], BF, name=f"o_{w_name}_{m}",
                                       tag=f"{tp}_{m}", bufs=1)
                    for ns in range(nsub):
                        c0, c1 = ns * 512, min((ns + 1) * 512, n_tok)
                        ps = pp.tile([128, c1 - c0], F32, tag="ps")
                        for k in range(FT):
                            nc.tensor.matmul(ps[:], w(k, m * 128, (m + 1) * 128),
                                             act_tiles[k][:, c0:c1],
                                             start=(k == 0), stop=(k == FT - 1))
                        if scale is not None:
                            nc.vector.tensor_scalar(ot[:, c0:c1], ps[:], scale,
                                                    bias_col(bias_name, m), MULT, ADD)
                        else:
                            nc.scalar.activation(ot[:, c0:c1], ps[:], IDENT,
                                                 bias=bias_col(bias_name, m))
                    outs.append(ot)
            return outs

        def proj_tm(w_name, act_tiles, bias_bcast, out_pool, tag_prefix=None):
            """token-major V tiles [KT x [128, H*65]] bf16 = act.T @ W + b, with
            a ones column appended after each head's 64 dims so the AV matmul's
            65th output row is the softmax denominator for free."""
            outs = []
            tp = tag_prefix or f"o_{w_name}"
            with tc.tile_pool(name=f"w_{w_name}", bufs=1) as wp, \
                 tc.tile_pool(name=f"ps_{w_name}", bufs=4, space=PSUM) as pp:
                w = load_w(w_name, wp, FT, D, parts=2)
                for m in range(KT):
                    ot = out_pool.tile([128, H * 65], BF, name=f"o_{w_name}_{m}",
                                       tag=f"{tp}_{m}", bufs=1)
                    ones_cols = ot.rearrange("p (h c) -> p h c", c=65)[:, :, 64:65]
                    nc.gpsimd.memset(ones_cols, 1.0)
                    for ns in range(2):
                        c0, c1 = ns * 512, (ns + 1) * 512
                        ps = pp.tile([128, 512], F32, tag="ps")
                        for k in range(FT):
                            nc.tensor.matmul(ps[:], act_tiles[k][:, m * 128:(m + 1) * 128],
                                             w(k, c0, c1),
                                             start=(k == 0), stop=(k == FT - 1))
                        dst = ot[:, ns * 8 * 65:(ns * 8 + 8) * 65].rearrange(
                            "p (h c) -> p h c", c=65)[:, :, 0:64]
                        psv = ps.rearrange("p (h c) -> p h c", c=64)
                        if bias_bcast is None:
                            copy_out(dst, psv, m * 2 + ns)
                        else:
                            bbv = bias_bcast[:, c0:c1].rearrange("p (h c) -> p h c", c=64)
                            nc.vector.scalar_tensor_tensor(dst, psv, 1.0, bbv, MULT, ADD)
                    outs.append(ot)
            return outs

        def attention(q_pairs, k_tiles, v_tiles, mask_tiles, tag, out_pool):
            """q_pairs feature-major [FT x [128,TOK]] bf16; k_tiles [FT x [128,S]];
            v_tiles token-major [KT x [128,H*65]]; mask_tiles bf16 doubled
            [KT x [128,2*TOK]] or None. Two heads of a feature pair share one
            2-bank scoresT psum tile (one matmul group per bank). The additive
            mask joins each head's accumulation group as an identity-weight
            matmul, so exp reads PSUM directly."""
            outs = []
            with tc.tile_pool(name=f"exp_{tag}", bufs=4) as epool, \
                 tc.tile_pool(name=f"asm_{tag}", bufs=4) as spool, \
                 tc.tile_pool(name=f"sT_{tag}", bufs=3, space=PSUM) as sps, \
                 tc.tile_pool(name=f"av_{tag}", bufs=2, space=PSUM) as avs:
                for hp in range(FT):
                    at = out_pool.tile([128, TOK], BF, name=f"at_{tag}_{hp}",
                                       tag=f"at_{hp}", bufs=1)
                    av0 = avs.tile([65, TOK], F32, tag="av")
                    av1 = avs.tile([65, TOK], F32, tag="av")
                    for kt in range(KT):
                        # two heads' scoresT in one 2-bank psum tile (one matmul
                        # group per bank); mask folded into each group via an
                        # identity-weight matmul
                        sTp = sps.tile([128, 4 * TOK], F32, tag="sT")
                        for hh in range(2):
                            po = hh * 64
                            nc.tensor.matmul(
                                sTp[:, hh * 2 * TOK:hh * 2 * TOK + TOK],
                                k_tiles[hp][po:po + 64, kt * 128:(kt + 1) * 128],
                                q_pairs[hp][po:po + 64, :], start=True,
                                stop=(mask_tiles is None))
                            if mask_tiles is not None:
                                nc.tensor.matmul(
                                    sTp[:, hh * 2 * TOK:hh * 2 * TOK + TOK],
                                    ident_sb[:],
                                    mask_tiles[kt][:, hh * TOK:(hh + 1) * TOK],
                                    start=False, stop=True)
                        sview = sTp.rearrange("p (b c) -> p b c", c=2 * TOK)[:, :, 0:TOK]
                        ex = epool.tile([128, 2 * TOK], BF, tag="exp")
                        exv = ex.rearrange("p (b c) -> p b c", c=TOK)
                        nc.scalar.activation(exv, sview, mybir.ActivationFunctionType.Exp)
                        for hh, av in ((0, av0), (1, av1)):
                            h = 2 * hp + hh
                            nc.tensor.matmul(av[:],
                                             v_tiles[kt][:, h * 65:(h + 1) * 65],
                                             ex[:, hh * TOK:(hh + 1) * TOK],
                                             start=(kt == 0), stop=(kt == KT - 1))
                    for hh, av in ((0, av0), (1, av1)):
                        rec = spool.tile([1, TOK], BF, tag="recip")
                        with nc.allow_low_precision(reason="softmax 1/sum broadcast in bf16"):
                            nc.vector.reciprocal(rec[:], av[64:65, :])
                        rb = sps.tile([64, TOK], F32, tag="sT")
                        nc.tensor.matmul(rb[:], ones_row_bf[0:1, 0:64],
                                         rec[:], start=True, stop=True)
                        rb_sb = spool.tile([64, TOK], F32, tag="rb_sb")
                        nc.vector.tensor_copy(rb_sb[:], rb[:])
                        nc.vector.tensor_mul(at[hh * 64:hh * 64 + 64, :], av[0:64, :], rb_sb[:])
                    outs.append(at)
            return outs

        def o_proj_residual(w_name, attn_tiles, bo_name, resid_tiles, rpool):
            outs = []
            with tc.tile_pool(name=f"w_{w_name}", bufs=1) as wp, \
                 tc.tile_pool(name=f"ps_{w_name}", bufs=4, space=PSUM) as pp:
                w = load_w(w_name, wp, FT, D, parts=2)
                for m in range(FT):
                    ps = pp.tile([128, TOK], F32, tag="ps")
                    for k in range(FT):
                        nc.tensor.matmul(ps[:], w(k, m * 128, (m + 1) * 128),
                                         attn_tiles[k][:], start=(k == 0), stop=(k == FT - 1))
                    rt = rpool.tile([128, TOK], F32, name=f"r_{w_name}_{m}", tag=f"r{m}")
                    nc.vector.scalar_tensor_tensor(rt[:], ps[:], bias_col(bo_name, m),
                                                   resid_tiles[m][:], ADD, ADD)
                    outs.append(rt)
            return outs

        def layer_norm(r_tiles, g_name, b_name, out_dtype, out_pool, want_bf16):
            with tc.tile_pool(name=f"lnp_{g_name}", bufs=1, space=PSUM) as lnps, \
                 tc.tile_pool(name=f"lnb_{g_name}", bufs=1, space=PSUM) as lnbc, \
                 tc.tile_pool(name=f"lns_{g_name}", bufs=2) as lnsm, \
                 tc.tile_pool(name=f"lnq_{g_name}", bufs=3) as sqp:
                s1 = lnps.tile([1, TOK], F32, tag="s1")
                s2 = lnps.tile([1, TOK], F32, tag="s2")
                for k in range(FT):
                    nc.tensor.matmul(s1[:], ones_f32[:], r_tiles[k][:],
                                     start=(k == 0), stop=(k == FT - 1))
                for k in range(FT):
                    sq = sqp.tile([128, TOK], F32, tag="sq")
                    nc.vector.tensor_mul(sq[:], r_tiles[k][:], r_tiles[k][:])
                    nc.tensor.matmul(s2[:], ones_f32[:], sq[:],
                                     start=(k == 0), stop=(k == FT - 1))
                mean = lnsm.tile([1, TOK], F32, tag="mean")
                nc.vector.tensor_scalar_mul(mean[:], s1[:], 1.0 / D)
                var = lnsm.tile([1, TOK], F32, tag="var")
                # var = s2/D - mean^2  ==  (s2 * 1/D) + (-mean*mean)
                nc.vector.scalar_tensor_tensor(var[:], mean[:], -1.0, mean[:], MULT, MULT)
                nc.vector.scalar_tensor_tensor(var[:], s2[:], 1.0 / D, var[:], MULT, ADD)
                nc.vector.tensor_scalar_add(var[:], var[:], 1e-5)
                std = lnsm.tile([1, TOK], F32, tag="std")
                nc.scalar.activation(std[:], var[:], mybir.ActivationFunctionType.Sqrt)
                # broadcasts run as bf16 matmuls (1 cycle/row vs 4 for fp32);
                # mean/rstd are small relative corrections so bf16 is plenty
                mean_bf = lnsm.tile([1, TOK], BF, tag="mean_bf")
                nc.vector.tensor_copy(mean_bf[:], mean[:])
                rstd_bf = lnsm.tile([1, TOK], BF, tag="rstd_bf")
                with nc.allow_low_precision(reason="LN rstd broadcast in bf16"):
                    nc.vector.reciprocal(rstd_bf[:], std[:])
                mean_b = lnbc.tile([128, TOK], F32, tag="meanb")
                nc.tensor.matmul(mean_b[:], ones_row_bf[:], mean_bf[:],
                                 start=True, stop=True)
                rstd_b = lnbc.tile([128, TOK], F32, tag="rstdb")
                nc.tensor.matmul(rstd_b[:], ones_row_bf[:], rstd_bf[:],
                                 start=True, stop=True)
                outs, outs_bf = [], []
                for k in range(FT):
                    xn = sqp.tile([128, TOK], F32, tag="xn")
                    nc.vector.tensor_sub(xn[:], r_tiles[k][:], mean_b[:])
                    nc.vector.tensor_mul(xn[:], xn[:], rstd_b[:])
                    ot = out_pool.tile([128, TOK], out_dtype, name=f"ln_{g_name}_{k}",
                                       tag=f"ln_{k}", bufs=1)
                    nc.vector.tensor_scalar(ot[:], xn[:], bias_col(g_name, k),
                                            bias_col(b_name, k), MULT, ADD)
                    outs.append(ot)
                    if want_bf16:
                        ob = out_pool.tile([128, TOK], BF, name=f"lnb_{g_name}_{k}",
                                           tag=f"lnbf_{k}", bufs=1)
                        nc.scalar.activation(ob[:], ot[:], IDENT)
                        outs_bf.append(ob)
            return outs, outs_bf

        # ================= phases 1-3: attention blocks =====================
        # One shared kv pool: cross K/V reuse self K/V buffers (same tags) —
        # self K/V are dead once self-attention completes.
        with tc.tile_pool(name="a2", bufs=FT) as a2pool:
            with tc.tile_pool(name="a1", bufs=FT) as a1pool, \
                 tc.tile_pool(name="kv", bufs=1) as kv_pool:
                # phase 1: Q first (x0cb bf16 is a direct input so the
                # first matmul only waits on 2.5MB of DMA)
                with tc.tile_pool(name="x0cb", bufs=1) as xcb_pool:
                    x0cb_b = xcb_pool.tile([128, FT, TOK], BF, name="x0cb_b", tag="xb")
                    nc.sync.dma_start(x0cb_b[:], g['x0cb'].rearrange("(a p) t -> p a t", p=128)[:])
                    x0cb = [x0cb_b[:, k, :] for k in range(FT)]
                    q_self = proj_fm('sWq', x0cb, TOK, 'sbq', kv_pool, scale=0.125)
                with tc.tile_pool(name="acts_x0", bufs=1) as actp:
                    x0_b = actp.tile([128, FT, S], BF, name="x0_b", tag="x0")
                    x0r = g['x0fm'].rearrange("(a p) t -> p a t", p=128)
                    nc.sync.dma_start(x0_b[:, 0:4, :], x0r[:, 0:4, :])
                    nc.sync.dma_start(x0_b[:, 4:8, :], x0r[:, 4:8, :])
                    x0_t = [x0_b[:, k, :] for k in range(FT)]
                    k_self = proj_fm('sWk', x0_t, S, 'sbk', kv_pool, tag_prefix="o_K")
                    v_self = proj_tm('sWv', x0_t, free_bias['sbv'], kv_pool, tag_prefix="o_V")

                # phase 2: self attention + O-proj + LN1
                with tc.tile_pool(name="x0c", bufs=1) as x0c_pool, \
                     tc.tile_pool(name="mask", bufs=1) as mask_pool, \
                     tc.tile_pool(name="r1p", bufs=1) as r1_pool:
                    maskT_t = None
                    if g['maskT'] is not None:
                        mt_b = mask_pool.tile([128, KT, 2 * TOK], BF, name="mt_b", tag="mt")
                        nc.sync.dma_start(mt_b[:], g['maskT'].rearrange("(a p) t -> p a t", p=128)[:])
                        maskT_t = [mt_b[:, k, :] for k in range(KT)]
                    x0c_b = x0c_pool.tile([128, FT, TOK], F32, name="x0c_b", tag="x0c")
                    nc.sync.dma_start(x0c_b[:], g['x0chunk'].rearrange("(a p) t -> p a t", p=128)[:])
                    x0c_t = [x0c_b[:, k, :] for k in range(FT)]

                    with tc.tile_pool(name="at_s", bufs=1) as at_pool_s:
                        attn1 = attention(q_self, k_self, v_self, maskT_t, "s", at_pool_s)
                        r1 = o_proj_residual('sWo', attn1, 'sbo', x0c_t, r1_pool)
                    a1, a1b = layer_norm(r1, 'ln1_g', 'ln1_b', F32, a1pool, True)

                with tc.tile_pool(name="acts_enc", bufs=1) as actp:
                    enc_b = actp.tile([128, FT, S], BF, name="enc_b", tag="enc")
                    encr = g['encfm'].rearrange("(a p) t -> p a t", p=128)
                    nc.sync.dma_start(enc_b[:, 0:4, :], encr[:, 0:4, :])
                    nc.sync.dma_start(enc_b[:, 4:8, :], encr[:, 4:8, :])
                    enc_t = [enc_b[:, k, :] for k in range(FT)]
                    k_cross = proj_fm('cWk', enc_t, S, 'cbk', kv_pool, tag_prefix="o_K")
                    v_cross = proj_tm('cWv', enc_t, free_bias['cbv'], kv_pool, tag_prefix="o_V")

                # phase 3: cross attention + O-proj + LN2
                with tc.tile_pool(name="qc", bufs=1) as qc_pool, \
                     tc.tile_pool(name="maskc", bufs=1) as maskc_pool, \
                     tc.tile_pool(name="r2p", bufs=1) as r2_pool:
                    maskTc_t = None
                    if g['maskTc'] is not None:
                        mtc_b = maskc_pool.tile([128, KT, 2 * TOK], BF, name="mtc_b", tag="mtc")
                        nc.sync.dma_start(mtc_b[:], g['maskTc'].rearrange("(a p) t -> p a t", p=128)[:])
                        maskTc_t = [mtc_b[:, k, :] for k in range(KT)]
                    q_cross = proj_fm('cWq', a1b, TOK, 'cbq', qc_pool, scale=0.125)
                    with tc.tile_pool(name="at_c", bufs=1) as at_pool_c:
                        attn2 = attention(q_cross, k_cross, v_cross, maskTc_t, "c", at_pool_c)
                        r2 = o_proj_residual('cWo', attn2, 'cbo', a1, r2_pool)
                    a2, a2b = layer_norm(r2, 'ln2_g', 'ln2_b', F32, a2pool, True)

            # ================= phase 4: FFN + LN3 (chunked weights) =========
            with tc.tile_pool(name="r3p", bufs=1) as r3_pool:
                r3 = [r3_pool.tile([128, TOK], F32, name=f"r_ffn_{m}", tag=f"r{m}")
                      for m in range(FT)]
                with tc.tile_pool(name="w_f1", bufs=2) as wp1, \
                     tc.tile_pool(name="w_f2", bufs=2) as wp2, \
                     tc.tile_pool(name="hid", bufs=2) as hpool, \
                     tc.tile_pool(name="ps_f1", bufs=3, space=PSUM) as pp1, \
                     tc.tile_pool(name="ps_f2", bufs=3, space=PSUM) as pp2:
                    f1r = g['fW1'].rearrange("(a p) d -> p a d", p=128)
                    f2r = g['fW2'].rearrange("(a p) d -> p a d", p=128)
                    for ci in range(NCH):
                        w1c = wp1.tile([128, FT, CHT * 128], BF, tag="w1")
                        nc.sync.dma_start(w1c[:], f1r[:, :, ci * CHT * 128:(ci + 1) * CHT * 128])
                        w2c = wp2.tile([128, CHT, D], BF, tag="w2")
                        nc.sync.dma_start(w2c[:], f2r[:, ci * CHT:(ci + 1) * CHT, :])
                        hbuf = hpool.tile([128, CHT, TOK], BF, tag="h")
                        for mh in range(CHT):
                            ps = pp1.tile([128, TOK], F32, tag="ps")
                            for k in range(FT):
                                nc.tensor.matmul(ps[:], w1c[:, k, mh * 128:(mh + 1) * 128],
                                                 a2b[k][:], start=(k == 0), stop=(k == FT - 1))
                            nc.scalar.activation(hbuf[:, mh, :], ps[:],
                                                 mybir.ActivationFunctionType.Relu,
                                                 bias=fb1_sb[:, ci * CHT + mh:ci * CHT + mh + 1])
                        for m in range(FT):
                            ps = pp2.tile([128, TOK], F32, tag="ps")
                            for kh in range(CHT):
                                nc.tensor.matmul(ps[:], w2c[:, kh, m * 128:(m + 1) * 128],
                                                 hbuf[:, kh, :], start=(kh == 0), stop=(kh == CHT - 1))
                            if ci == 0:
                                nc.vector.scalar_tensor_tensor(r3[m][:], ps[:], bias_col('fb2', m),
                                                               a2[m][:], ADD, ADD)
                            else:
                                nc.vector.tensor_add(r3[m][:], r3[m][:], ps[:])
                y, _ = layer_norm(r3, 'ln3_g', 'ln3_b', BF, ypool, False)

            # Wout ring prefetch: queued behind the FFN weight DMAs, streams
            # during FFN/LN3 compute.
            for n in range(RING):
                emit_wo_load(n)

        # ================= phase 5: vocab projection ========================
        with tc.tile_pool(name="vout", bufs=4) as vos, \
             tc.tile_pool(name="vps", bufs=4, space=PSUM) as vps:
            for n in range(VN):
                wt = wo_tiles.pop(n)
                for m in range(2):
                    ps = vps.tile([128, VC], F32, tag="ps")
                    for k in range(FT):
                        nc.tensor.matmul(ps[:], y[k][:, m * 128:(m + 1) * 128],
                                         wt[:, k, :], start=(k == 0), stop=(k == FT - 1))
                    ot = vos.tile([128, VC], BF, tag="vo")
                    copy_out(ot[:], ps[:], n * 2 + m)
                    nc.gpsimd.dma_start(g['out'][m * 128:(m + 1) * 128, n * VC:(n + 1) * VC], ot[:])
                if n + RING < VN:
                    emit_wo_load(n + RING)


def host_prep(inputs):
    x0 = np.asarray(inputs['dec_input'], np.float32) + positional_encoding(S, D)[None]
    enc = np.asarray(inputs['enc_input'], np.float32)
    mask_self = np.asarray(inputs['masked_attention_mask'], np.float32)[0, 0]
    mask_cross = np.asarray(inputs['cross_attention_mask'], np.float32)[0, 0]
    self_adds = bool(np.any(mask_self != 0.0))
    cross_adds = bool(np.any(mask_cross != 0.0))
    li = L - 1
    Wl = {}
    for p in ['sWq', 'sWk', 'sWv', 'sWo', 'cWq', 'cWk', 'cWv', 'cWo', 'fW1', 'fW2']:
        Wl[p] = np.ascontiguousarray(np.asarray(inputs[p], np.float32)[li]).astype(BF16)
    bv = {}
    for p in ['sbq', 'sbk', 'sbv', 'sbo', 'cbq', 'cbk', 'cbv', 'cbo',
              'ln1_g', 'ln1_b', 'ln2_g', 'ln2_b', 'ln3_g', 'ln3_b', 'fb1', 'fb2']:
        bv[p] = np.asarray(inputs[p], np.float32)[li]
    Wout_bf = np.ascontiguousarray(np.asarray(inputs['Wout'], np.float32)).astype(BF16)
    bout = np.asarray(inputs['bout'], np.float32)
    ident = np.eye(128, dtype=BF16)

    def pp(v):  # [1024] -> [128, 8] partition-major
        return np.ascontiguousarray(v.reshape(-1, 128).T)

    bias_cols = []
    for name in BIAS_NAMES:
        src = {'sbq': bv['sbq'] * 0.125, 'cbq': bv['cbq'] * 0.125}.get(name, bv.get(name))
        bias_cols.append(pp(src))
    biases_pp = np.ascontiguousarray(np.concatenate(bias_cols, axis=1), np.float32)
    fb1_pp = np.ascontiguousarray(bv['fb1'].reshape(HT, 128).T, np.float32)

    in_maps = []
    for core in range(NC):
        b, c = core // 4, core % 4
        q0 = c * TOK
        x0c = np.ascontiguousarray(x0[b, q0:q0 + TOK].T)
        m = {
            'x0fm': np.ascontiguousarray(x0[b].T).astype(BF16),
            'encfm': np.ascontiguousarray(enc[b].T).astype(BF16),
            'x0chunk': np.ascontiguousarray(x0c, np.float32),
            'x0cb': x0c.astype(BF16),
            'biases': biases_pp, 'fb1': fb1_pp, 'ident': ident,
            'sbv_row': np.ascontiguousarray(bv['sbv'][None, :], np.float32),
            'cbv_row': np.ascontiguousarray(bv['cbv'][None, :], np.float32),
            'Wout': Wout_bf,
        }
        m.update(Wl)
        if self_adds:
            mt = mask_self[q0:q0 + TOK, :].T
            m['maskT'] = np.ascontiguousarray(np.concatenate([mt, mt], axis=1)).astype(BF16)
        if cross_adds:
            mt = mask_cross[q0:q0 + TOK, :].T
            m['maskTc'] = np.ascontiguousarray(np.concatenate([mt, mt], axis=1)).astype(BF16)
        in_maps.append(m)
    zero_free = not (np.any(bv['sbv']) or np.any(bv['cbv']))
    return in_maps, self_adds, cross_adds, zero_free, bout


_CACHE = {}


def _get_program(self_adds, cross_adds, zero_free):
    key = (self_adds, cross_adds, zero_free)
    if key not in _CACHE:
        _CACHE[key] = build_program(self_adds, cross_adds, zero_free)
    return _CACHE[key]


def kernel(**inputs):
    in_maps, self_adds, cross_adds, zero_free, bout = host_prep(inputs)
    nc = _get_program(self_adds, cross_adds, zero_free)
    res = run_bass_kernel_spmd(nc, in_maps, core_ids=list(range(NC)))
    shards = [np.asarray(res.results[r]["out"], dtype=np.float32) for r in range(NC)]
    full = np.concatenate(shards, axis=0)           # [2048, V]
    if np.any(bout):
        full = full + bout[None, :]
    return np.ascontiguousarray(full.reshape(B, S, V), np.float32)


# revision 33
# speedup vs baseline: 1.3445x; 1.0656x over previous
"""Trainium2 Bass kernel for nn_Decoder_3539053052044.

Structure (v2 — no collectives):
- The reference decoder has a preserved bug: every layer consumes the ORIGINAL
  x0, so only the LAST layer's output survives. We compute layer L-1 only.
- Sequence-parallel: 8 cores x 256 tokens (core r -> batch r//4, chunk r%4).
  Each core computes the full last layer for its 256 tokens (K/V projections
  for its whole batch are computed locally), then projects its OWN 256 tokens
  against the FULL vocab (full Wout streamed from HBM in 500-col chunks
  through an 8-deep SBUF ring) — zero cross-core communication.
- Activations are feature-major [D on partitions, tokens free] so every linear
  layer uses the stored [D_in, D_out] weights directly as lhsT.
- Softmax is max-free (scores are O(1) for this model; exp(-1e9)=0 handles
  masking) and computed directly transposed, scoresT[k,q]. The additive mask
  is folded into the scores PSUM accumulation group via an identity-weight
  matmul (PE), so exp reads PSUM directly — no DVE mask-add, no SBUF staging.
  V carries a ones column per head ([128, H*65]) so the AV matmul's 65th
  output row is the softmax denominator for free.
- LayerNorm / broadcast matmuls run as float32r (1 cycle/row at free>=256 in
  the PE, vs 4 for plain fp32).
- FFN weights are streamed in 4 hidden-chunks (2MB each, double buffered) with
  the fW2 contraction accumulated into SBUF, freeing SBUF for the Wout ring.
- Output is written bf16 ([256, 32000] per core) and converted to fp32 on the
  host; vocab bias (all-zero here) would be added host-side.
- bf16 matmul inputs, fp32 accumulation.
"""

import numpy as np
import ml_dtypes

import concourse.bass as bass
import concourse.bacc as bacc
import concourse.tile as tile
from concourse import mybir
from concourse.bass_utils import run_bass_kernel_spmd
from concourse.vector_clock import ScopedClock, VectorClock

BF16 = ml_dtypes.bfloat16
F32 = mybir.dt.float32
F32R = mybir.dt.float32r
BF = mybir.dt.bfloat16
PSUM = bass.MemorySpace.PSUM

B, S, D, H, L, V, DF = 2, 1024, 1024, 16, 4, 32000, 4096
DH = D // H              # 64
NC = 8                   # cores
TOK = B * S // NC        # 256 tokens per core
KT = S // 128            # 8 k tiles
FT = D // 128            # 8 feature tiles
HT = DF // 128           # 32 hidden tiles
NCH = 8                  # FFN hidden chunks
CHT = HT // NCH          # 4 hidden tiles per chunk
VC = 500                 # vocab cols per chunk
VN = V // VC             # 64 vocab chunks
RING = 9                 # Wout ring depth
ADD = mybir.AluOpType.add
MULT = mybir.AluOpType.mult
IDENT = mybir.ActivationFunctionType.Identity
ADD_OP = mybir.AluOpType.add

_PATCHED = False


def _patch_tile_drain():
    """This neuronxcc build rejects a Drain carrying >1 sem wait. Split the
    Tile tail drain into one Drain per busy proc, each with a single wait."""
    global _PATCHED
    if _PATCHED:
        return
    _PATCHED = True

    def _drain_and_barrier_split(self, tick_clock, wait_clock):
        gc = tick_clock.global_clock
        n = len(gc)
        for p in range(n):
            if gc[p] > 0:
                vc = VectorClock([gc[q] if q == p else 0 for q in range(n)])
                d = self.nc.sync.drain()
                wait_clock.add_sem_waits(d.ins, ScopedClock({None: vc}))
        self.nc.sync.drain()
        self.nc.all_engine_barrier()
        assert self.sems is not None
        popped = self.nc._tile_sem_poison_stack.pop()
        assert popped is self._sem_poison
        self.nc.clear_and_free_semaphores(list(self.sems.allocated().values()))
        self.nc.all_engine_barrier()

    tile.TileContext._drain_and_barrier = _drain_and_barrier_split


def positional_encoding(seq_len, d_model, n=10000.0):
    i = np.arange(seq_len, dtype=np.float32)[:, None]
    d = np.arange(d_model)
    denom = np.power(n, (2 * (d // 2)).astype(np.float32) / d_model)
    ang = i / denom
    return np.where(d % 2 == 0, np.sin(ang), np.cos(ang)).astype(np.float32)


BIAS_NAMES = ['sbk', 'sbq', 'sbo', 'cbk', 'cbq', 'cbo', 'fb2',
              'ln1_g', 'ln1_b', 'ln2_g', 'ln2_b', 'ln3_g', 'ln3_b']


def build_program(self_mask_adds: bool, cross_mask_adds: bool, zero_free_biases: bool = False,
                  unit_ln: bool = False):
    _patch_tile_drain()
    nc = bacc.Bacc()

    g = {}  # dram handles
    g['x0fm'] = nc.declare_dram_parameter("x0fm", [D, S], BF, isOutput=False)
    g['encfm'] = nc.declare_dram_parameter("encfm", [D, S], BF, isOutput=False)
    g['x0chunk'] = nc.declare_dram_parameter("x0chunk", [D, TOK], F32, isOutput=False)
    g['x0cb'] = nc.declare_dram_parameter("x0cb", [D, TOK], BF, isOutput=False)
    for w in ['sWq', 'sWk', 'sWv', 'sWo', 'cWq', 'cWk', 'cWv', 'cWo']:
        g[w] = nc.declare_dram_parameter(w, [D, D], BF, isOutput=False)
    g['fW1'] = nc.declare_dram_parameter("fW1", [D, DF], BF, isOutput=False)
    g['fW2'] = nc.declare_dram_parameter("fW2", [DF, D], BF, isOutput=False)
    g['Wout'] = nc.declare_dram_parameter("Wout", [D, V], BF, isOutput=False)
    g['biases'] = nc.declare_dram_parameter("biases", [128, 8 * len(BIAS_NAMES)], F32, isOutput=False)
    g['fb1'] = nc.declare_dram_parameter("fb1", [128, HT], F32, isOutput=False)
    g['ident'] = nc.declare_dram_parameter("ident", [128, 128], BF, isOutput=False)
    g['sbv_row'] = nc.declare_dram_parameter("sbv_row", [1, D], F32, isOutput=False)
    g['cbv_row'] = nc.declare_dram_parameter("cbv_row", [1, D], F32, isOutput=False)
    g['maskT'] = nc.declare_dram_parameter("maskT", [S, TOK], BF, isOutput=False) if self_mask_adds else None
    g['maskTc'] = nc.declare_dram_parameter("maskTc", [S, TOK], BF, isOutput=False) if cross_mask_adds else None
    g['out'] = nc.declare_dram_parameter("out", [TOK, V], BF, isOutput=True)

    with tile.TileContext(nc) as tc:
        _emit(nc, tc, g, zero_free_biases, unit_ln)
    nc.compile()
    return nc


def _emit(nc, tc, g, zero_free_biases, unit_ln):
    from contextlib import ExitStack
    ctx = ExitStack()
    with ctx:
        # ---------- whole-kernel constants / small tensors ------------------
        const = ctx.enter_context(tc.tile_pool(name="const", bufs=1))
        ones_bf = const.tile([128, 1], BF, name="ones_bf", tag="c0")
        nc.gpsimd.memset(ones_bf[:], 1.0)
        ones_f32 = const.tile([128, 1], F32, name="ones_f32", tag="c1")
        nc.gpsimd.memset(ones_f32[:], 1.0)
        ones_row = const.tile([1, 128], F32, name="ones_row", tag="c2")
        nc.gpsimd.memset(ones_row[:], 1.0)
        ones_row_bf = const.tile([1, 128], BF, name="ones_row_bf", tag="c6")
        nc.gpsimd.memset(ones_row_bf[:], 1.0)
        # const DMAs are emitted in phase 1 after the x0cb/sWq loads (they are
        # not needed until the first bias add) so the first matmul starts early
        bias_sb = const.tile([128, 8 * len(BIAS_NAMES)], F32, name="bias_sb", tag="c3")
        fb1_sb = const.tile([128, HT], F32, name="fb1_sb", tag="c4")
        ident_sb = const.tile([128, 128], BF, name="ident_sb", tag="c5")

        def emit_const_dmas():
            nc.sync.dma_start(bias_sb[:], g['biases'][:])
            nc.sync.dma_start(fb1_sb[:], g['fb1'][:])
            nc.sync.dma_start(ident_sb[:], g['ident'][:])

        def bias_col(name, f):
            i = BIAS_NAMES.index(name)
            return bias_sb[:, i * 8 + f:i * 8 + f + 1]

        # free-axis bias broadcast tiles [128, D] for sbv / cbv (skipped when
        # the host observed all-zero free-axis biases)
        free_bias = {'sbv': None, 'cbv': None}
        if not zero_free_biases:
            with tc.tile_pool(name="bbc_ps", bufs=1, space=PSUM) as bps, \
                 tc.tile_pool(name="bbc_row", bufs=2) as brow:
                for bi, bname in enumerate(['sbv', 'cbv']):
                    t = const.tile([128, D], F32, name=f"{bname}_b", tag=f"fb{bi}")
                    rsb = brow.tile([1, D], F32, tag="row")
                    nc.sync.dma_start(rsb[:], g[f'{bname}_row'][:])
                    for half in range(2):
                        ps = bps.tile([128, 512], F32, tag="bc")
                        nc.tensor.matmul(ps[:], ones_row[:],
                                         rsb[0:1, half * 512:(half + 1) * 512],
                                         start=True, stop=True)
                        nc.vector.tensor_copy(t[:, half * 512:(half + 1) * 512], ps[:])
                    free_bias[bname] = t

        # Wout ring: reserved at the bottom of the SBUF stack for the whole
        # program so the prefetch DMAs never anti-depend on phase pools.
        wo_pool = ctx.enter_context(tc.tile_pool(name="wo_ring", bufs=RING))
        wo_tiles = {}

        def emit_wo_load(n):
            t = wo_pool.tile([128, FT, VC], BF, name=f"wo_{n}", tag="wo")
            nc.sync.dma_start(
                t[:], g['Wout'].rearrange("(a p) v -> p a v", p=128)[:, :, n * VC:(n + 1) * VC])
            wo_tiles[n] = t

        def copy_out(ot, ps, idx):
            """psum->sbuf copy alternating DVE/ACT to balance engines"""
            if idx % 2 == 0:
                nc.vector.tensor_copy(ot, ps)
            else:
                nc.scalar.activation(ot, ps, IDENT)

        def load_w(w_name, pool, kt_n, cols, parts=1):
            """Load [kt_n*128, cols] weight as `parts` batched tiles.
            Returns accessor: wslice(k, c0, c1) -> [128, c1-c0] lhsT/rhs AP."""
            per = kt_n // parts
            tiles = []
            for pi in range(parts):
                wt = pool.tile([128, per, cols], BF, name=f"w_{w_name}_{pi}", tag=f"w{pi}")
                nc.sync.dma_start(
                    wt[:], g[w_name].rearrange("(a p) d -> p a d", p=128)[:, pi * per:(pi + 1) * per, :])
                tiles.append(wt)

            def wslice(k, c0, c1):
                return tiles[k // per][:, k % per, c0:c1]
            return wslice

        # LN outputs: a1 lives through phase 3; a2 through FFN; y to the end.
        ypool = ctx.enter_context(tc.tile_pool(name="y", bufs=FT))

        # ---------- helpers -------------------------------------------------
        def proj_fm(w_name, act_tiles, n_tok, bias_name, out_pool, scale=None,
                    tag_prefix=None, w_acc=None):
            """feature-major out tiles [FT x [128, n_tok]] bf16 = W.T @ act + b"""
            outs = []
            tp = tag_prefix or f"o_{w_name}"
            nsub = (n_tok + 511) // 512
            from contextlib import nullcontext
            wctx = nullcontext() if w_acc else tc.tile_pool(name=f"w_{w_name}", bufs=1)
            with wctx as wp, \
                 tc.tile_pool(name=f"ps_{w_name}", bufs=4, space=PSUM) as pp:
                w = w_acc or load_w(w_name, wp, FT, D, parts=2)
                for m in range(FT):
                    ot = out_pool.tile([128, n_tok], BF, name=f"o_{w_name}_{m}",
                                       tag=f"{tp}_{m}", bufs=1)
                    for ns in range(nsub):
                        c0, c1 = ns * 512, min((ns + 1) * 512, n_tok)
                        ps = pp.tile([128, c1 - c0], F32, tag="ps")
                        for k in range(FT):
                            nc.tensor.matmul(ps[:], w(k, m * 128, (m + 1) * 128),
                                             act_tiles[k][:, c0:c1],
                                             start=(k == 0), stop=(k == FT - 1))
                        if scale is not None:
                            nc.vector.tensor_scalar(ot[:, c0:c1], ps[:], scale,
                                                    bias_col(bias_name, m), MULT, ADD)
                        elif (m + ns) % 2 == 0:
                            nc.vector.tensor_scalar(ot[:, c0:c1], ps[:], 1.0,
                                                    bias_col(bias_name, m), MULT, ADD)
                        else:
                            nc.scalar.activation(ot[:, c0:c1], ps[:], IDENT,
                                                 bias=bias_col(bias_name, m))
                    outs.append(ot)
            return outs

        def proj_fm_ko(w, act_tiles, n_tok, bias_name, out_pool, scale=None,
                       tag_prefix="o"):
            """k-outer feature-major projection: all FT output psums accumulate
            simultaneously (8 banks) so compute starts as soon as the first
            weight k-tile and activation k-tile arrive."""
            outs = []
            nsub = (n_tok + 511) // 512
            with tc.tile_pool(name=f"ps_{tag_prefix}", bufs=1, space=PSUM) as pp:
                for m in range(FT):
                    outs.append(out_pool.tile([128, n_tok], BF, name=f"{tag_prefix}{m}",
                                              tag=f"{tag_prefix}_{m}", bufs=1))
                for ns in range(nsub):
                    c0, c1 = ns * 512, min((ns + 1) * 512, n_tok)
                    pss = [pp.tile([128, c1 - c0], F32, name=f"ps{m}", tag=f"ps{m}") for m in range(FT)]
                    for k in range(FT):
                        for m in range(FT):
                            nc.tensor.matmul(pss[m][:], w(k, m * 128, (m + 1) * 128),
                                             act_tiles[k][:, c0:c1],
                                             start=(k == 0), stop=(k == FT - 1))
                    for m in range(FT):
                        if scale is not None:
                            nc.vector.tensor_scalar(outs[m][:, c0:c1], pss[m][:], scale,
                                                    bias_col(bias_name, m), MULT, ADD)
                        elif m % 2 == 0:
                            nc.vector.tensor_scalar(outs[m][:, c0:c1], pss[m][:], 1.0,
                                                    bias_col(bias_name, m), MULT, ADD)
                        else:
                            nc.scalar.activation(outs[m][:, c0:c1], pss[m][:], IDENT,
                                                 bias=bias_col(bias_name, m))
            return outs

        def proj_tm_ko(w, act_tiles, bias_bcast, out_pool, tag_prefix="oV"):
            """k-outer token-major V projection with the ones-column trick."""
            outs = []
            with tc.tile_pool(name=f"ps_{tag_prefix}", bufs=1, space=PSUM) as pp:
                for m in range(KT):
                    ot = out_pool.tile([128, H * 65], BF, name=f"{tag_prefix}{m}",
                                       tag=f"{tag_prefix}_{m}", bufs=1)
                    ones_cols = ot.rearrange("p (h c) -> p h c", c=65)[:, :, 64:65]
                    nc.gpsimd.memset(ones_cols, 1.0)
                    outs.append(ot)
                for ns in range(2):
                    c0, c1 = ns * 512, (ns + 1) * 512
                    pss = [pp.tile([128, 512], F32, name=f"ps{m}", tag=f"ps{m}") for m in range(KT)]
                    for k in range(FT):
                        for m in range(KT):
                            nc.tensor.matmul(pss[m][:], act_tiles[k][:, m * 128:(m + 1) * 128],
                                             w(k, c0, c1), start=(k == 0), stop=(k == FT - 1))
                    for m in range(KT):
                        dst = outs[m][:, ns * 8 * 65:(ns * 8 + 8) * 65].rearrange(
                            "p (h c) -> p h c", c=65)[:, :, 0:64]
                        psv = pss[m].rearrange("p (h c) -> p h c", c=64)
                        if bias_bcast is None:
                            copy_out(dst, psv, m * 2 + ns)
                        else:
                            bbv = bias_bcast[:, c0:c1].rearrange("p (h c) -> p h c", c=64)
                            nc.vector.scalar_tensor_tensor(dst, psv, 1.0, bbv, MULT, ADD)
            return outs

        def proj_tm(w_name, act_tiles, bias_bcast, out_pool, tag_prefix=None):
            """token-major V tiles [KT x [128, H*65]] bf16 = act.T @ W + b, with
            a ones column appended after each head's 64 dims so the AV matmul's
            65th output row is the softmax denominator for free."""
            outs = []
            tp = tag_prefix or f"o_{w_name}"
            with tc.tile_pool(name=f"w_{w_name}", bufs=1) as wp, \
                 tc.tile_pool(name=f"ps_{w_name}", bufs=4, space=PSUM) as pp:
                w = load_w(w_name, wp, FT, D, parts=2)
                for m in range(KT):
                    ot = out_pool.tile([128, H * 65], BF, name=f"o_{w_name}_{m}",
                                       tag=f"{tp}_{m}", bufs=1)
                    ones_cols = ot.rearrange("p (h c) -> p h c", c=65)[:, :, 64:65]
                    nc.gpsimd.memset(ones_cols, 1.0)
                    for ns in range(2):
                        c0, c1 = ns * 512, (ns + 1) * 512
                        ps = pp.tile([128, 512], F32, tag="ps")
                        for k in range(FT):
                            nc.tensor.matmul(ps[:], act_tiles[k][:, m * 128:(m + 1) * 128],
                                             w(k, c0, c1),
                                             start=(k == 0), stop=(k == FT - 1))
                        dst = ot[:, ns * 8 * 65:(ns * 8 + 8) * 65].rearrange(
                            "p (h c) -> p h c", c=65)[:, :, 0:64]
                        psv = ps.rearrange("p (h c) -> p h c", c=64)
                        if bias_bcast is None:
                            copy_out(dst, psv, m * 2 + ns)
                        else:
                            bbv = bias_bcast[:, c0:c1].rearrange("p (h c) -> p h c", c=64)
                            nc.vector.scalar_tensor_tensor(dst, psv, 1.0, bbv, MULT, ADD)
                    outs.append(ot)
            return outs

        def attention(q_pairs, k_tiles, v_tiles, mask_tiles, tag, out_pool):
            """q_pairs feature-major [FT x [128,TOK]] bf16; k_tiles [FT x [128,S]];
            v_tiles token-major [KT x [128,H*65]]; mask_tiles bf16 doubled
            [KT x [128,2*TOK]] or None. Two heads of a feature pair share one
            2-bank scoresT psum tile (one matmul group per bank). The additive
            mask joins each head's accumulation group as an identity-weight
            matmul, so exp reads PSUM directly."""
            outs = []
            with tc.tile_pool(name=f"exp_{tag}", bufs=4) as epool, \
                 tc.tile_pool(name=f"asm_{tag}", bufs=4) as spool, \
                 tc.tile_pool(name=f"sT_{tag}", bufs=3, space=PSUM) as sps, \
                 tc.tile_pool(name=f"av_{tag}", bufs=2, space=PSUM) as avs:
                for hp in range(FT):
                    at = out_pool.tile([128, TOK], BF, name=f"at_{tag}_{hp}",
                                       tag=f"at_{hp}", bufs=1)
                    av0 = avs.tile([65, TOK], F32, tag="av")
                    av1 = avs.tile([65, TOK], F32, tag="av")
                    for kt in range(KT):
                        # two heads' scoresT in one 2-bank psum tile (one matmul
                        # group per bank); mask folded into each group via an
                        # identity-weight matmul
                        sTp = sps.tile([128, 4 * TOK], F32, tag="sT")
                        for hh in range(2):
                            po = hh * 64
                            nc.tensor.matmul(
                                sTp[:, hh * 2 * TOK:hh * 2 * TOK + TOK],
                                k_tiles[hp][po:po + 64, kt * 128:(kt + 1) * 128],
                                q_pairs[hp][po:po + 64, :], start=True,
                                stop=(mask_tiles is None))
                            if mask_tiles is not None:
                                nc.tensor.matmul(
                                    sTp[:, hh * 2 * TOK:hh * 2 * TOK + TOK],
                                    ident_sb[:], mask_tiles[kt][:],
                                    start=False, stop=True)
                        sview = sTp.rearrange("p (b c) -> p b c", c=2 * TOK)[:, :, 0:TOK]
                        ex = epool.tile([128, 2 * TOK], BF, tag="exp")
                        exv = ex.rearrange("p (b c) -> p b c", c=TOK)
                        nc.scalar.activation(exv, sview, mybir.ActivationFunctionType.Exp)
                        for hh, av in ((0, av0), (1, av1)):
                            h = 2 * hp + hh
                            nc.tensor.matmul(av[:],
                                             v_tiles[kt][:, h * 65:(h + 1) * 65],
                                             ex[:, hh * TOK:(hh + 1) * TOK],
                                             start=(kt == 0), stop=(kt == KT - 1))
                    for hh, av in ((0, av0), (1, av1)):
                        rec = spool.tile([1, TOK], BF, tag="recip")
                        with nc.allow_low_precision(reason="softmax 1/sum broadcast in bf16"):
                            nc.vector.reciprocal(rec[:], av[64:65, :])
                        rb = sps.tile([64, TOK], F32, tag="sT")
                        nc.tensor.matmul(rb[:], ones_row_bf[0:1, 0:64],
                                         rec[:], start=True, stop=True)
                        rb_sb = spool.tile([64, TOK], F32, tag="rb_sb")
                        nc.vector.tensor_copy(rb_sb[:], rb[:])
                        nc.vector.tensor_mul(at[hh * 64:hh * 64 + 64, :], av[0:64, :], rb_sb[:])
                    outs.append(at)
            return outs

        def o_proj_residual_ko(w, tag, attn_tiles, bo_name, resid_tiles, rpool):
            """k-outer O-projection + bias + residual: starts on attn tile 0
            while later head-pairs are still finishing."""
            outs = []
            with tc.tile_pool(name=f"ps_{tag}", bufs=1, space=PSUM) as pp:
                pss = [pp.tile([128, TOK], F32, name=f"ps{m}", tag=f"ps{m}") for m in range(FT)]
                for k in range(FT):
                    for m in range(FT):
                        nc.tensor.matmul(pss[m][:], w(k, m * 128, (m + 1) * 128),
                                         attn_tiles[k][:], start=(k == 0), stop=(k == FT - 1))
                for m in range(FT):
                    rt = rpool.tile([128, TOK], F32, name=f"r_{tag}_{m}", tag=f"r{m}")
                    nc.vector.scalar_tensor_tensor(rt[:], pss[m][:], bias_col(bo_name, m),
                                                   resid_tiles[m][:], ADD, ADD)
                    outs.append(rt)
            return outs

        def layer_norm(r_tiles, g_name, b_name, out_dtype, out_pool, want_bf16,
                       interleave=None):
            inter_result = None
            with tc.tile_pool(name=f"lnp_{g_name}", bufs=1, space=PSUM) as lnps, \
                 tc.tile_pool(name=f"lnb_{g_name}", bufs=1, space=PSUM) as lnbc, \
                 tc.tile_pool(name=f"lns_{g_name}", bufs=1) as lnsm, \
                 tc.tile_pool(name=f"lnq_{g_name}", bufs=2) as sqp:
                s1 = lnps.tile([1, TOK], F32, tag="s1")
                s2 = lnps.tile([1, TOK], F32, tag="s2")
                for k in range(FT):
                    nc.tensor.matmul(s1[:], ones_f32[:], r_tiles[k][:],
                                     start=(k == 0), stop=(k == FT - 1))
                for k in range(FT):
                    sq = sqp.tile([128, TOK], F32, tag="sq")
                    nc.vector.tensor_mul(sq[:], r_tiles[k][:], r_tiles[k][:])
                    nc.tensor.matmul(s2[:], ones_f32[:], sq[:],
                                     start=(k == 0), stop=(k == FT - 1))
                mean = lnsm.tile([1, TOK], F32, tag="mean")
                nc.vector.tensor_scalar_mul(mean[:], s1[:], 1.0 / D)
                var = lnsm.tile([1, TOK], F32, tag="var")
                # var = s2/D - mean^2  ==  (s2 * 1/D) + (-mean*mean)
                nc.vector.scalar_tensor_tensor(var[:], mean[:], -1.0, mean[:], MULT, MULT)
                nc.vector.scalar_tensor_tensor(var[:], s2[:], 1.0 / D, var[:], MULT, ADD)
                nc.vector.tensor_scalar_add(var[:], var[:], 1e-5)
                std = lnsm.tile([1, TOK], F32, tag="std")
                nc.scalar.activation(std[:], var[:], mybir.ActivationFunctionType.Sqrt)
                # broadcasts run as bf16 matmuls (1 cycle/row vs 4 for fp32);
                # mean/rstd are small relative corrections so bf16 is plenty
                mean_bf = lnsm.tile([1, TOK], BF, tag="mean_bf")
                nc.vector.tensor_copy(mean_bf[:], mean[:])
                rstd_bf = lnsm.tile([1, TOK], BF, tag="rstd_bf")
                with nc.allow_low_precision(reason="LN rstd broadcast in bf16"):
                    nc.vector.reciprocal(rstd_bf[:], std[:])
                mean_b = lnbc.tile([128, TOK], F32, tag="meanb")
                nc.tensor.matmul(mean_b[:], ones_row_bf[:], mean_bf[:],
                                 start=True, stop=True)
                rstd_b = lnbc.tile([128, TOK], F32, tag="rstdb")
                nc.tensor.matmul(rstd_b[:], ones_row_bf[:], rstd_bf[:],
                                 start=True, stop=True)
                if interleave is not None:
                    # emit independent PE work (cross K projection) here so the
                    # tensor engine streams while the LN tail runs on DVE
                    inter_result = interleave()
                outs, outs_bf = [], []
                for k in range(FT):
                    ot = out_pool.tile([128, TOK], out_dtype, name=f"ln_{g_name}_{k}",
                                       tag=f"ln_{k}", bufs=1)
                    if unit_ln:
                        # gamma==1, beta==0: out = (r - mean) * rstd in 2 DVE ops
                        xn = sqp.tile([128, TOK], F32, tag="xn")
                        nc.vector.tensor_sub(xn[:], r_tiles[k][:], mean_b[:])
                        nc.vector.tensor_mul(ot[:], xn[:], rstd_b[:])
                    else:
                        xn = sqp.tile([128, TOK], F32, tag="xn")
                        nc.vector.tensor_sub(xn[:], r_tiles[k][:], mean_b[:])
                        nc.vector.tensor_mul(xn[:], xn[:], rstd_b[:])
                        nc.vector.tensor_scalar(ot[:], xn[:], bias_col(g_name, k),
                                                bias_col(b_name, k), MULT, ADD)
                    outs.append(ot)
                    if want_bf16:
                        ob = out_pool.tile([128, TOK], BF, name=f"lnb_{g_name}_{k}",
                                           tag=f"lnbf_{k}", bufs=1)
                        nc.scalar.activation(ob[:], ot[:], IDENT)
                        outs_bf.append(ob)
            return outs, outs_bf, inter_result

        # ================= phases 1-3: attention blocks =====================
        # One shared kv pool: cross K/V reuse self K/V buffers (same tags) —
        # self K/V are dead once self-attention completes.
        def load_w_parts(w_name, pool, parts=8):
            """Emit `parts` DMAs for a [D, cols] weight; returns accessor."""
            cols = g[w_name].shape[1]
            per = FT // parts
            tiles = []
            for pi in range(parts):
                wt = pool.tile([128, per, cols], BF, name=f"w_{w_name}_{pi}", tag=f"w{pi}")
                nc.sync.dma_start(
                    wt[:], g[w_name].rearrange("(a p) d -> p a d", p=128)[:, pi * per:(pi + 1) * per, :])
                tiles.append(wt)

            def wslice(k, c0, c1):
                return tiles[k // per][:, k % per, c0:c1]
            return wslice

        with tc.tile_pool(name="a2", bufs=FT) as a2pool:
            with tc.tile_pool(name="a1", bufs=FT) as a1pool, \
                 tc.tile_pool(name="kv", bufs=1) as kv_pool, \
                 tc.tile_pool(name="mask", bufs=1) as mask_pool:
                # phase 1: Q first (x0cb bf16 is a direct input so the
                # first matmul only waits on 2.5MB of DMA)
                with tc.tile_pool(name="x0cb", bufs=1) as xcb_pool, \
                     tc.tile_pool(name="w_sWq", bufs=1) as wq_pool:
                    x0cb_b = xcb_pool.tile([128, FT, TOK], BF, name="x0cb_b", tag="xb")
                    nc.sync.dma_start(x0cb_b[:], g['x0cb'].rearrange("(a p) t -> p a t", p=128)[:])
                    x0cb = [x0cb_b[:, k, :] for k in range(FT)]
                    wq = load_w_parts('sWq', wq_pool, parts=8)
                    maskT_t = None
                    if g['maskT'] is not None:
                        mt_b = mask_pool.tile([128, KT, TOK], BF, name="mt_b", tag="mt")
                        nc.sync.dma_start(mt_b[:], g['maskT'].rearrange("(a p) t -> p a t", p=128)[:])
                        maskT_t = [mt_b[:, k, :] for k in range(KT)]
                    q_self = proj_fm_ko(wq, x0cb, TOK, 'sbq', kv_pool, scale=0.125,
                                        tag_prefix="o_Q")
                with tc.tile_pool(name="acts_x0", bufs=1) as actp, \
                     tc.tile_pool(name="w_sWk", bufs=1) as wk_pool, \
                     tc.tile_pool(name="w_sWv", bufs=1) as wv_pool:
                    # interleave x0 k-tile and sWk part DMAs so the k-outer
                    # K projection starts after ~1.5MB of DMA
                    x0r = g['x0fm'].rearrange("(a p) t -> p a t", p=128)
                    wkr = g['sWk'].rearrange("(a p) d -> p a d", p=128)
                    x0_t, wk_tiles = [], []
                    for k in range(FT):
                        xt = actp.tile([128, S], BF, name=f"x0_{k}", tag=f"x0_{k}")
                        nc.sync.dma_start(xt[:], x0r[:, k, :])
                        x0_t.append(xt)
                        wt = wk_pool.tile([128, 1, D], BF, name=f"w_sWk_{k}", tag=f"w{k}")
                        nc.sync.dma_start(wt[:], wkr[:, k:k + 1, :])
                        wk_tiles.append(wt)
                    wk = lambda k, c0, c1: wk_tiles[k][:, 0, c0:c1]
                    wv = load_w_parts('sWv', wv_pool, parts=8)
                    k_self = proj_fm_ko(wk, x0_t, S, 'sbk', kv_pool, tag_prefix="o_K")
                    v_self = proj_tm_ko(wv, x0_t, free_bias['sbv'], kv_pool, tag_prefix="o_V")

                # phase 2: self attention + O-proj + LN1.  enc / sWo / x0c
                # DMAs are emitted before the attention body so they stream
                # during attention compute.
                with tc.tile_pool(name="acts_enc", bufs=1) as encp, \
                     tc.tile_pool(name="x0c", bufs=1) as x0c_pool, \
                     tc.tile_pool(name="r1p", bufs=1) as r1_pool:
                    encr = g['encfm'].rearrange("(a p) t -> p a t", p=128)
                    enc_t = []
                    for k in range(FT):
                        et = encp.tile([128, S], BF, name=f"enc_{k}", tag=f"enc_{k}")
                        nc.sync.dma_start(et[:], encr[:, k, :])
                        enc_t.append(et)
                    x0c_b = x0c_pool.tile([128, FT, TOK], F32, name="x0c_b", tag="x0c")
                    nc.sync.dma_start(x0c_b[:], g['x0chunk'].rearrange("(a p) t -> p a t", p=128)[:])
                    x0c_t = [x0c_b[:, k, :] for k in range(FT)]

                    with tc.tile_pool(name="w_sWo", bufs=1) as wo_pool_s:
                        swo = load_w_parts('sWo', wo_pool_s, parts=2)
                        with tc.tile_pool(name="at_s", bufs=1) as at_pool_s:
                            attn1 = attention(q_self, k_self, v_self, maskT_t, "s", at_pool_s)
                        r1 = o_proj_residual_ko(swo, "so", attn1, 'sbo', x0c_t, r1_pool)

                    with tc.tile_pool(name="w_cWk", bufs=1) as wck_pool:
                        cwk = load_w_parts('cWk', wck_pool, parts=8)

                        def inter1():
                            return proj_fm('cWk', enc_t, S, 'cbk', kv_pool,
                                           tag_prefix="o_K", w_acc=cwk)
                        a1, a1b, k_cross = layer_norm(r1, 'ln1_g', 'ln1_b', F32,
                                                      a1pool, True, interleave=inter1)
                    v_cross = proj_tm('cWv', enc_t, free_bias['cbv'], kv_pool,
                                      tag_prefix="o_V")

                # phase 3: cross attention + O-proj + LN2
                with tc.tile_pool(name="qc", bufs=1) as qc_pool, \
                     tc.tile_pool(name="maskc", bufs=1) as maskc_pool, \
                     tc.tile_pool(name="w_cWo", bufs=1) as wo_pool_c, \
                     tc.tile_pool(name="r2p", bufs=1) as r2_pool:
                    maskTc_t = None
                    if g['maskTc'] is not None:
                        mtc_b = maskc_pool.tile([128, KT, TOK], BF, name="mtc_b", tag="mtc")
                        nc.sync.dma_start(mtc_b[:], g['maskTc'].rearrange("(a p) t -> p a t", p=128)[:])
                        maskTc_t = [mtc_b[:, k, :] for k in range(KT)]
                    q_cross = proj_fm('cWq', a1b, TOK, 'cbq', qc_pool, scale=0.125)
                    cwo = load_w_parts('cWo', wo_pool_c, parts=2)
                    with tc.tile_pool(name="at_c", bufs=1) as at_pool_c:
                        attn2 = attention(q_cross, k_cross, v_cross, maskTc_t, "c", at_pool_c)
                        r2 = o_proj_residual_ko(cwo, "co", attn2, 'cbo', a1, r2_pool)
                    a2, a2b, _ = layer_norm(r2, 'ln2_g', 'ln2_b', F32, a2pool, True)

            # ================= phase 4: FFN + LN3 (chunked weights) =========
            with tc.tile_pool(name="r3p", bufs=1) as r3_pool:
                r3 = [r3_pool.tile([128, TOK], F32, name=f"r_ffn_{m}", tag=f"r{m}")
                      for m in range(FT)]
                with tc.tile_pool(name="w_f1", bufs=2) as wp1, \
                     tc.tile_pool(name="w_f2", bufs=2) as wp2, \
                     tc.tile_pool(name="hid", bufs=2) as hpool, \
                     tc.tile_pool(name="ps_f1", bufs=3, space=PSUM) as pp1, \
                     tc.tile_pool(name="ps_f2", bufs=3, space=PSUM) as pp2:
                    f1r = g['fW1'].rearrange("(a p) d -> p a d", p=128)
                    f2r = g['fW2'].rearrange("(a p) d -> p a d", p=128)
                    for ci in range(NCH):
                        w1c = wp1.tile([128, FT, CHT * 128], BF, tag="w1")
                        nc.sync.dma_start(w1c[:], f1r[:, :, ci * CHT * 128:(ci + 1) * CHT * 128])
                        w2c = wp2.tile([128, CHT, D], BF, tag="w2")
                        nc.sync.dma_start(w2c[:], f2r[:, ci * CHT:(ci + 1) * CHT, :])
                        hbuf = hpool.tile([128, CHT, TOK], BF, tag="h")
                        for mh in range(CHT):
                            ps = pp1.tile([128, TOK], F32, tag="ps")
                            for k in range(FT):
                                nc.tensor.matmul(ps[:], w1c[:, k, mh * 128:(mh + 1) * 128],
                                                 a2b[k][:], start=(k == 0), stop=(k == FT - 1))
                            nc.scalar.activation(hbuf[:, mh, :], ps[:],
                                                 mybir.ActivationFunctionType.Relu,
                                                 bias=fb1_sb[:, ci * CHT + mh:ci * CHT + mh + 1])
                        for m in range(FT):
                            ps = pp2.tile([128, TOK], F32, tag="ps")
                            for kh in range(CHT):
                                nc.tensor.matmul(ps[:], w2c[:, kh, m * 128:(m + 1) * 128],
                                                 hbuf[:, kh, :], start=(kh == 0), stop=(kh == CHT - 1))
                            if ci == 0:
                                nc.vector.scalar_tensor_tensor(r3[m][:], ps[:], bias_col('fb2', m),
                                                               a2[m][:], ADD, ADD)
                            else:
                                nc.vector.tensor_add(r3[m][:], r3[m][:], ps[:])
                y, _, _ = layer_norm(r3, 'ln3_g', 'ln3_b', BF, ypool, False)

            # Wout ring prefetch: queued behind the FFN weight DMAs, streams
            # during FFN/LN3 compute.
            for n in range(RING):
                emit_wo_load(n)

        # ================= phase 5: vocab projection ========================
        with tc.tile_pool(name="vout", bufs=4) as vos, \
             tc.tile_pool(name="vps", bufs=4, space=PSUM) as vps:
            for n in range(VN):
                wt = wo_tiles.pop(n)
                for m in range(2):
                    ps = vps.tile([128, VC], F32, tag="ps")
                    for k in range(FT):
                        nc.tensor.matmul(ps[:], y[k][:, m * 128:(m + 1) * 128],
                                         wt[:, k, :], start=(k == 0), stop=(k == FT - 1))
                    ot = vos.tile([128, VC], BF, tag="vo")
                    copy_out(ot[:], ps[:], n * 2 + m)
                    nc.gpsimd.dma_start(g['out'][m * 128:(m + 1) * 128, n * VC:(n + 1) * VC], ot[:])
                if n + RING < VN:
                    emit_wo_load(n + RING)


def host_prep(inputs):
    x0 = np.asarray(inputs['dec_input'], np.float32) + positional_encoding(S, D)[None]
    enc = np.asarray(inputs['enc_input'], np.float32)
    mask_self = np.asarray(inputs['masked_attention_mask'], np.float32)[0, 0]
    mask_cross = np.asarray(inputs['cross_attention_mask'], np.float32)[0, 0]
    self_adds = bool(np.any(mask_self != 0.0))
    cross_adds = bool(np.any(mask_cross != 0.0))
    li = L - 1
    Wl = {}
    for p in ['sWq', 'sWk', 'sWv', 'sWo', 'cWq', 'cWk', 'cWv', 'cWo', 'fW1', 'fW2']:
        Wl[p] = np.ascontiguousarray(np.asarray(inputs[p], np.float32)[li]).astype(BF16)
    bv = {}
    for p in ['sbq', 'sbk', 'sbv', 'sbo', 'cbq', 'cbk', 'cbv', 'cbo',
              'ln1_g', 'ln1_b', 'ln2_g', 'ln2_b', 'ln3_g', 'ln3_b', 'fb1', 'fb2']:
        bv[p] = np.asarray(inputs[p], np.float32)[li]
    Wout_bf = np.ascontiguousarray(np.asarray(inputs['Wout'], np.float32)).astype(BF16)
    bout = np.asarray(inputs['bout'], np.float32)
    ident = np.eye(128, dtype=BF16)

    def pp(v):  # [1024] -> [128, 8] partition-major
        return np.ascontiguousarray(v.reshape(-1, 128).T)

    bias_cols = []
    for name in BIAS_NAMES:
        src = {'sbq': bv['sbq'] * 0.125, 'cbq': bv['cbq'] * 0.125}.get(name, bv.get(name))
        bias_cols.append(pp(src))
    biases_pp = np.ascontiguousarray(np.concatenate(bias_cols, axis=1), np.float32)
    fb1_pp = np.ascontiguousarray(bv['fb1'].reshape(HT, 128).T, np.float32)

    in_maps = []
    for core in range(NC):
        b, c = core // 4, core % 4
        q0 = c * TOK
        x0c = np.ascontiguousarray(x0[b, q0:q0 + TOK].T)
        m = {
            'x0fm': np.ascontiguousarray(x0[b].T).astype(BF16),
            'encfm': np.ascontiguousarray(enc[b].T).astype(BF16),
            'x0chunk': np.ascontiguousarray(x0c, np.float32),
            'x0cb': x0c.astype(BF16),
            'biases': biases_pp, 'fb1': fb1_pp, 'ident': ident,
            'sbv_row': np.ascontiguousarray(bv['sbv'][None, :], np.float32),
            'cbv_row': np.ascontiguousarray(bv['cbv'][None, :], np.float32),
            'Wout': Wout_bf,
        }
        m.update(Wl)
        if self_adds:
            m['maskT'] = np.ascontiguousarray(mask_self[q0:q0 + TOK, :].T).astype(BF16)
        if cross_adds:
            m['maskTc'] = np.ascontiguousarray(mask_cross[q0:q0 + TOK, :].T).astype(BF16)
        in_maps.append(m)
    zero_free = not (np.any(bv['sbv']) or np.any(bv['cbv']))
    unit_ln = all(np.all(bv[f'ln{i}_g'] == 1.0) and not np.any(bv[f'ln{i}_b'])
                  for i in (1, 2, 3))
    return in_maps, self_adds, cross_adds, zero_free, unit_ln, bout


_CACHE = {}


def _get_program(self_adds, cross_adds, zero_free, unit_ln):
    key = (self_adds, cross_adds, zero_free, unit_ln)
    if key not in _CACHE:
        _CACHE[key] = build_program(self_adds, cross_adds, zero_free, unit_ln)
    return _CACHE[key]


def kernel(**inputs):
    in_maps, self_adds, cross_adds, zero_free, unit_ln, bout = host_prep(inputs)
    nc = _get_program(self_adds, cross_adds, zero_free, unit_ln)
    res = run_bass_kernel_spmd(nc, in_maps, core_ids=list(range(NC)))
    shards = [np.asarray(res.results[r]["out"], dtype=np.float32) for r in range(NC)]
    full = np.concatenate(shards, axis=0)           # [2048, V]
    if np.any(bout):
        full = full + bout[None, :]
    return np.ascontiguousarray(full.reshape(B, S, V), np.float32)


# revision 34
# speedup vs baseline: 1.3735x; 1.0215x over previous
"""Trainium2 Bass kernel for nn_Decoder_3539053052044.

Structure (v2 — no collectives):
- The reference decoder has a preserved bug: every layer consumes the ORIGINAL
  x0, so only the LAST layer's output survives. We compute layer L-1 only.
- Sequence-parallel: 8 cores x 256 tokens (core r -> batch r//4, chunk r%4).
  Each core computes the full last layer for its 256 tokens (K/V projections
  for its whole batch are computed locally), then projects its OWN 256 tokens
  against the FULL vocab (full Wout streamed from HBM in 500-col chunks
  through an 8-deep SBUF ring) — zero cross-core communication.
- Activations are feature-major [D on partitions, tokens free] so every linear
  layer uses the stored [D_in, D_out] weights directly as lhsT.
- Softmax is max-free (scores are O(1) for this model; exp(-1e9)=0 handles
  masking) and computed directly transposed, scoresT[k,q]. The additive mask
  is folded into the scores PSUM accumulation group via an identity-weight
  matmul (PE), so exp reads PSUM directly — no DVE mask-add, no SBUF staging.
  V carries a ones column per head ([128, H*65]) so the AV matmul's 65th
  output row is the softmax denominator for free.
- LayerNorm / broadcast matmuls run as float32r (1 cycle/row at free>=256 in
  the PE, vs 4 for plain fp32).
- FFN weights are streamed in 4 hidden-chunks (2MB each, double buffered) with
  the fW2 contraction accumulated into SBUF, freeing SBUF for the Wout ring.
- Output is written bf16 ([256, 32000] per core) and converted to fp32 on the
  host; vocab bias (all-zero here) would be added host-side.
- bf16 matmul inputs, fp32 accumulation.
"""

import numpy as np
import ml_dtypes

import concourse.bass as bass
import concourse.bacc as bacc
import concourse.tile as tile
from concourse import mybir
from concourse.bass_utils import run_bass_kernel_spmd
from concourse.vector_clock import ScopedClock, VectorClock

BF16 = ml_dtypes.bfloat16
F32 = mybir.dt.float32
F32R = mybir.dt.float32r
BF = mybir.dt.bfloat16
PSUM = bass.MemorySpace.PSUM

B, S, D, H, L, V, DF = 2, 1024, 1024, 16, 4, 32000, 4096
DH = D // H              # 64
NC = 8                   # cores
TOK = B * S // NC        # 256 tokens per core
KT = S // 128            # 8 k tiles
FT = D // 128            # 8 feature tiles
HT = DF // 128           # 32 hidden tiles
NCH = 8                  # FFN hidden chunks
CHT = HT // NCH          # 4 hidden tiles per chunk
VC = 500                 # vocab cols per chunk
VN = V // VC             # 64 vocab chunks
RING = 9                 # Wout ring depth
ADD = mybir.AluOpType.add
MULT = mybir.AluOpType.mult
IDENT = mybir.ActivationFunctionType.Identity
ADD_OP = mybir.AluOpType.add

_PATCHED = False


def _patch_tile_drain():
    """This neuronxcc build rejects a Drain carrying >1 sem wait. Split the
    Tile tail drain into one Drain per busy proc, each with a single wait."""
    global _PATCHED
    if _PATCHED:
        return
    _PATCHED = True

    def _drain_and_barrier_split(self, tick_clock, wait_clock):
        gc = tick_clock.global_clock
        n = len(gc)
        for p in range(n):
            if gc[p] > 0:
                vc = VectorClock([gc[q] if q == p else 0 for q in range(n)])
                d = self.nc.sync.drain()
                wait_clock.add_sem_waits(d.ins, ScopedClock({None: vc}))
        self.nc.sync.drain()
        self.nc.all_engine_barrier()
        assert self.sems is not None
        popped = self.nc._tile_sem_poison_stack.pop()
        assert popped is self._sem_poison
        self.nc.clear_and_free_semaphores(list(self.sems.allocated().values()))
        self.nc.all_engine_barrier()

    tile.TileContext._drain_and_barrier = _drain_and_barrier_split


def positional_encoding(seq_len, d_model, n=10000.0):
    i = np.arange(seq_len, dtype=np.float32)[:, None]
    d = np.arange(d_model)
    denom = np.power(n, (2 * (d // 2)).astype(np.float32) / d_model)
    ang = i / denom
    return np.where(d % 2 == 0, np.sin(ang), np.cos(ang)).astype(np.float32)


BIAS_NAMES = ['sbk', 'sbq', 'sbo', 'cbk', 'cbq', 'cbo', 'fb2',
              'ln1_g', 'ln1_b', 'ln2_g', 'ln2_b', 'ln3_g', 'ln3_b']


def build_program(self_mask_adds: bool, cross_mask_adds: bool, zero_free_biases: bool = False,
                  unit_ln: bool = False):
    _patch_tile_drain()
    nc = bacc.Bacc()

    g = {}  # dram handles
    g['x0fm'] = nc.declare_dram_parameter("x0fm", [D, S], BF, isOutput=False)
    g['encfm'] = nc.declare_dram_parameter("encfm", [D, S], BF, isOutput=False)
    g['x0chunk'] = nc.declare_dram_parameter("x0chunk", [D, TOK], F32, isOutput=False)
    g['x0cb'] = nc.declare_dram_parameter("x0cb", [D, TOK], BF, isOutput=False)
    for w in ['sWq', 'sWk', 'sWv', 'sWo', 'cWq', 'cWk', 'cWv', 'cWo']:
        g[w] = nc.declare_dram_parameter(w, [D, D], BF, isOutput=False)
    g['fW1'] = nc.declare_dram_parameter("fW1", [D, DF], BF, isOutput=False)
    g['fW2'] = nc.declare_dram_parameter("fW2", [DF, D], BF, isOutput=False)
    g['Wout'] = nc.declare_dram_parameter("Wout", [D, V], BF, isOutput=False)
    g['biases'] = nc.declare_dram_parameter("biases", [128, 8 * len(BIAS_NAMES)], F32, isOutput=False)
    g['fb1'] = nc.declare_dram_parameter("fb1", [128, HT], F32, isOutput=False)
    g['ident'] = nc.declare_dram_parameter("ident", [128, 128], BF, isOutput=False)
    g['sbv_row'] = nc.declare_dram_parameter("sbv_row", [1, D], F32, isOutput=False)
    g['cbv_row'] = nc.declare_dram_parameter("cbv_row", [1, D], F32, isOutput=False)
    g['maskT'] = nc.declare_dram_parameter("maskT", [S, TOK], BF, isOutput=False) if self_mask_adds else None
    g['maskTc'] = nc.declare_dram_parameter("maskTc", [S, TOK], BF, isOutput=False) if cross_mask_adds else None
    g['out'] = nc.declare_dram_parameter("out", [TOK, V], BF, isOutput=True)

    with tile.TileContext(nc) as tc:
        _emit(nc, tc, g, zero_free_biases, unit_ln)
    nc.compile()
    return nc


def _emit(nc, tc, g, zero_free_biases, unit_ln):
    from contextlib import ExitStack
    ctx = ExitStack()
    with ctx:
        # ---------- whole-kernel constants / small tensors ------------------
        const = ctx.enter_context(tc.tile_pool(name="const", bufs=1))
        ones_bf = const.tile([128, 1], BF, name="ones_bf", tag="c0")
        nc.gpsimd.memset(ones_bf[:], 1.0)
        ones_f32 = const.tile([128, 1], F32, name="ones_f32", tag="c1")
        nc.gpsimd.memset(ones_f32[:], 1.0)
        ones_row = const.tile([1, 128], F32, name="ones_row", tag="c2")
        nc.gpsimd.memset(ones_row[:], 1.0)
        ones_row_bf = const.tile([1, 128], BF, name="ones_row_bf", tag="c6")
        nc.gpsimd.memset(ones_row_bf[:], 1.0)
        # const DMAs are emitted in phase 1 after the x0cb/sWq loads (they are
        # not needed until the first bias add) so the first matmul starts early
        bias_sb = const.tile([128, 8 * len(BIAS_NAMES)], F32, name="bias_sb", tag="c3")
        fb1_sb = const.tile([128, HT], F32, name="fb1_sb", tag="c4")
        ident_sb = const.tile([128, 128], BF, name="ident_sb", tag="c5")

        def emit_const_dmas():
            nc.sync.dma_start(bias_sb[:], g['biases'][:])
            nc.sync.dma_start(fb1_sb[:], g['fb1'][:])
            nc.sync.dma_start(ident_sb[:], g['ident'][:])

        def bias_col(name, f):
            i = BIAS_NAMES.index(name)
            return bias_sb[:, i * 8 + f:i * 8 + f + 1]

        # free-axis bias broadcast tiles [128, D] for sbv / cbv (skipped when
        # the host observed all-zero free-axis biases)
        free_bias = {'sbv': None, 'cbv': None}
        if not zero_free_biases:
            with tc.tile_pool(name="bbc_ps", bufs=1, space=PSUM) as bps, \
                 tc.tile_pool(name="bbc_row", bufs=2) as brow:
                for bi, bname in enumerate(['sbv', 'cbv']):
                    t = const.tile([128, D], F32, name=f"{bname}_b", tag=f"fb{bi}")
                    rsb = brow.tile([1, D], F32, tag="row")
                    nc.sync.dma_start(rsb[:], g[f'{bname}_row'][:])
                    for half in range(2):
                        ps = bps.tile([128, 512], F32, tag="bc")
                        nc.tensor.matmul(ps[:], ones_row[:],
                                         rsb[0:1, half * 512:(half + 1) * 512],
                                         start=True, stop=True)
                        nc.vector.tensor_copy(t[:, half * 512:(half + 1) * 512], ps[:])
                    free_bias[bname] = t

        # Wout ring: reserved at the bottom of the SBUF stack for the whole
        # program so the prefetch DMAs never anti-depend on phase pools.
        wo_pool = ctx.enter_context(tc.tile_pool(name="wo_ring", bufs=RING))
        wo_tiles = {}

        def emit_wo_load(n):
            t = wo_pool.tile([128, FT, VC], BF, name=f"wo_{n}", tag="wo")
            nc.sync.dma_start(
                t[:], g['Wout'].rearrange("(a p) v -> p a v", p=128)[:, :, n * VC:(n + 1) * VC])
            wo_tiles[n] = t

        def copy_out(ot, ps, idx):
            """psum->sbuf copy alternating DVE/ACT to balance engines"""
            if idx % 2 == 0:
                nc.vector.tensor_copy(ot, ps)
            else:
                nc.scalar.activation(ot, ps, IDENT)

        def load_w(w_name, pool, kt_n, cols, parts=1):
            """Load [kt_n*128, cols] weight as `parts` batched tiles.
            Returns accessor: wslice(k, c0, c1) -> [128, c1-c0] lhsT/rhs AP."""
            per = kt_n // parts
            tiles = []
            for pi in range(parts):
                wt = pool.tile([128, per, cols], BF, name=f"w_{w_name}_{pi}", tag=f"w{pi}")
                nc.sync.dma_start(
                    wt[:], g[w_name].rearrange("(a p) d -> p a d", p=128)[:, pi * per:(pi + 1) * per, :])
                tiles.append(wt)

            def wslice(k, c0, c1):
                return tiles[k // per][:, k % per, c0:c1]
            return wslice

        # LN outputs: a1 lives through phase 3; a2 through FFN; y to the end.
        ypool = ctx.enter_context(tc.tile_pool(name="y", bufs=FT))

        # ---------- helpers -------------------------------------------------
        def proj_fm(w_name, act_tiles, n_tok, bias_name, out_pool, scale=None,
                    tag_prefix=None, w_acc=None):
            """feature-major out tiles [FT x [128, n_tok]] bf16 = W.T @ act + b"""
            outs = []
            tp = tag_prefix or f"o_{w_name}"
            nsub = (n_tok + 511) // 512
            from contextlib import nullcontext
            wctx = nullcontext() if w_acc else tc.tile_pool(name=f"w_{w_name}", bufs=1)
            with wctx as wp, \
                 tc.tile_pool(name=f"ps_{w_name}", bufs=4, space=PSUM) as pp:
                w = w_acc or load_w(w_name, wp, FT, D, parts=2)
                for m in range(FT):
                    ot = out_pool.tile([128, n_tok], BF, name=f"o_{w_name}_{m}",
                                       tag=f"{tp}_{m}", bufs=1)
                    for ns in range(nsub):
                        c0, c1 = ns * 512, min((ns + 1) * 512, n_tok)
                        ps = pp.tile([128, c1 - c0], F32, tag="ps")
                        for k in range(FT):
                            nc.tensor.matmul(ps[:], w(k, m * 128, (m + 1) * 128),
                                             act_tiles[k][:, c0:c1],
                                             start=(k == 0), stop=(k == FT - 1))
                        if scale is not None:
                            nc.vector.tensor_scalar(ot[:, c0:c1], ps[:], scale,
                                                    bias_col(bias_name, m), MULT, ADD)
                        elif (m + ns) % 2 == 0:
                            nc.vector.tensor_scalar(ot[:, c0:c1], ps[:], 1.0,
                                                    bias_col(bias_name, m), MULT, ADD)
                        else:
                            nc.scalar.activation(ot[:, c0:c1], ps[:], IDENT,
                                                 bias=bias_col(bias_name, m))
                    outs.append(ot)
            return outs

        def proj_fm_ko(w, act_tiles, n_tok, bias_name, out_pool, scale=None,
                       tag_prefix="o"):
            """k-outer feature-major projection: all FT output psums accumulate
            simultaneously (8 banks) so compute starts as soon as the first
            weight k-tile and activation k-tile arrive."""
            outs = []
            nsub = (n_tok + 511) // 512
            with tc.tile_pool(name=f"ps_{tag_prefix}", bufs=1, space=PSUM) as pp:
                for m in range(FT):
                    outs.append(out_pool.tile([128, n_tok], BF, name=f"{tag_prefix}{m}",
                                              tag=f"{tag_prefix}_{m}", bufs=1))
                for ns in range(nsub):
                    c0, c1 = ns * 512, min((ns + 1) * 512, n_tok)
                    pss = [pp.tile([128, c1 - c0], F32, name=f"ps{m}", tag=f"ps{m}") for m in range(FT)]
                    for k in range(FT):
                        for m in range(FT):
                            nc.tensor.matmul(pss[m][:], w(k, m * 128, (m + 1) * 128),
                                             act_tiles[k][:, c0:c1],
                                             start=(k == 0), stop=(k == FT - 1))
                    for m in range(FT):
                        if scale is not None:
                            nc.vector.tensor_scalar(outs[m][:, c0:c1], pss[m][:], scale,
                                                    bias_col(bias_name, m), MULT, ADD)
                        elif m % 2 == 0:
                            nc.vector.tensor_scalar(outs[m][:, c0:c1], pss[m][:], 1.0,
                                                    bias_col(bias_name, m), MULT, ADD)
                        else:
                            nc.scalar.activation(outs[m][:, c0:c1], pss[m][:], IDENT,
                                                 bias=bias_col(bias_name, m))
            return outs

        def proj_tm_ko(w, act_tiles, bias_bcast, out_pool, tag_prefix="oV"):
            """k-outer token-major V projection with the ones-column trick."""
            outs = []
            with tc.tile_pool(name=f"ps_{tag_prefix}", bufs=1, space=PSUM) as pp:
                for m in range(KT):
                    ot = out_pool.tile([128, H * 65], BF, name=f"{tag_prefix}{m}",
                                       tag=f"{tag_prefix}_{m}", bufs=1)
                    ones_cols = ot.rearrange("p (h c) -> p h c", c=65)[:, :, 64:65]
                    nc.gpsimd.memset(ones_cols, 1.0)
                    outs.append(ot)
                for ns in range(2):
                    c0, c1 = ns * 512, (ns + 1) * 512
                    pss = [pp.tile([128, 512], F32, name=f"ps{m}", tag=f"ps{m}") for m in range(KT)]
                    for k in range(FT):
                        for m in range(KT):
                            nc.tensor.matmul(pss[m][:], act_tiles[k][:, m * 128:(m + 1) * 128],
                                             w(k, c0, c1), start=(k == 0), stop=(k == FT - 1))
                    for m in range(KT):
                        dst = outs[m][:, ns * 8 * 65:(ns * 8 + 8) * 65].rearrange(
                            "p (h c) -> p h c", c=65)[:, :, 0:64]
                        psv = pss[m].rearrange("p (h c) -> p h c", c=64)
                        if bias_bcast is None:
                            copy_out(dst, psv, m * 2 + ns)
                        else:
                            bbv = bias_bcast[:, c0:c1].rearrange("p (h c) -> p h c", c=64)
                            nc.vector.scalar_tensor_tensor(dst, psv, 1.0, bbv, MULT, ADD)
            return outs

        def proj_tm(w_name, act_tiles, bias_bcast, out_pool, tag_prefix=None):
            """token-major V tiles [KT x [128, H*65]] bf16 = act.T @ W + b, with
            a ones column appended after each head's 64 dims so the AV matmul's
            65th output row is the softmax denominator for free."""
            outs = []
            tp = tag_prefix or f"o_{w_name}"
            with tc.tile_pool(name=f"w_{w_name}", bufs=1) as wp, \
                 tc.tile_pool(name=f"ps_{w_name}", bufs=4, space=PSUM) as pp:
                w = load_w(w_name, wp, FT, D, parts=2)
                for m in range(KT):
                    ot = out_pool.tile([128, H * 65], BF, name=f"o_{w_name}_{m}",
                                       tag=f"{tp}_{m}", bufs=1)
                    ones_cols = ot.rearrange("p (h c) -> p h c", c=65)[:, :, 64:65]
                    nc.gpsimd.memset(ones_cols, 1.0)
                    for ns in range(2):
                        c0, c1 = ns * 512, (ns + 1) * 512
                        ps = pp.tile([128, 512], F32, tag="ps")
                        for k in range(FT):
                            nc.tensor.matmul(ps[:], act_tiles[k][:, m * 128:(m + 1) * 128],
                                             w(k, c0, c1),
                                             start=(k == 0), stop=(k == FT - 1))
                        dst = ot[:, ns * 8 * 65:(ns * 8 + 8) * 65].rearrange(
                            "p (h c) -> p h c", c=65)[:, :, 0:64]
                        psv = ps.rearrange("p (h c) -> p h c", c=64)
                        if bias_bcast is None:
                            copy_out(dst, psv, m * 2 + ns)
                        else:
                            bbv = bias_bcast[:, c0:c1].rearrange("p (h c) -> p h c", c=64)
                            nc.vector.scalar_tensor_tensor(dst, psv, 1.0, bbv, MULT, ADD)
                    outs.append(ot)
            return outs

        def attention(q_pairs, k_tiles, v_tiles, mask_tiles, tag, out_pool):
            """q_pairs feature-major [FT x [128,TOK]] bf16; k_tiles [FT x [128,S]];
            v_tiles token-major [KT x [128,H*65]]; mask_tiles bf16 doubled
            [KT x [128,2*TOK]] or None. Two heads of a feature pair share one
            2-bank scoresT psum tile (one matmul group per bank). The additive
            mask joins each head's accumulation group as an identity-weight
            matmul, so exp reads PSUM directly."""
            outs = []
            with tc.tile_pool(name=f"exp_{tag}", bufs=4) as epool, \
                 tc.tile_pool(name=f"asm_{tag}", bufs=4) as spool, \
                 tc.tile_pool(name=f"sT_{tag}", bufs=3, space=PSUM) as sps, \
                 tc.tile_pool(name=f"av_{tag}", bufs=2, space=PSUM) as avs:
                for hp in range(FT):
                    at = out_pool.tile([128, TOK], BF, name=f"at_{tag}_{hp}",
                                       tag=f"at_{hp}", bufs=1)
                    av0 = avs.tile([65, TOK], F32, tag="av")
                    av1 = avs.tile([65, TOK], F32, tag="av")
                    for kt in range(KT):
                        # two heads' scoresT in one 2-bank psum tile (one matmul
                        # group per bank); mask folded into each group via an
                        # identity-weight matmul
                        sTp = sps.tile([128, 4 * TOK], F32, tag="sT")
                        for hh in range(2):
                            po = hh * 64
                            nc.tensor.matmul(
                                sTp[:, hh * 2 * TOK:hh * 2 * TOK + TOK],
                                k_tiles[hp][po:po + 64, kt * 128:(kt + 1) * 128],
                                q_pairs[hp][po:po + 64, :], start=True,
                                stop=(mask_tiles is None))
                            if mask_tiles is not None:
                                nc.tensor.matmul(
                                    sTp[:, hh * 2 * TOK:hh * 2 * TOK + TOK],
                                    ident_sb[:], mask_tiles[kt][:],
                                    start=False, stop=True)
                        sview = sTp.rearrange("p (b c) -> p b c", c=2 * TOK)[:, :, 0:TOK]
                        ex = epool.tile([128, 2 * TOK], BF, tag="exp")
                        exv = ex.rearrange("p (b c) -> p b c", c=TOK)
                        nc.scalar.activation(exv, sview, mybir.ActivationFunctionType.Exp)
                        for hh, av in ((0, av0), (1, av1)):
                            h = 2 * hp + hh
                            nc.tensor.matmul(av[:],
                                             v_tiles[kt][:, h * 65:(h + 1) * 65],
                                             ex[:, hh * TOK:(hh + 1) * TOK],
                                             start=(kt == 0), stop=(kt == KT - 1))
                    for hh, av in ((0, av0), (1, av1)):
                        rec = spool.tile([1, TOK], BF, tag="recip")
                        with nc.allow_low_precision(reason="softmax 1/sum broadcast in bf16"):
                            nc.vector.reciprocal(rec[:], av[64:65, :])
                        rb = sps.tile([64, TOK], F32, tag="sT")
                        nc.tensor.matmul(rb[:], ones_row_bf[0:1, 0:64],
                                         rec[:], start=True, stop=True)
                        rb_sb = spool.tile([64, TOK], F32, tag="rb_sb")
                        nc.vector.tensor_copy(rb_sb[:], rb[:])
                        nc.vector.tensor_mul(at[hh * 64:hh * 64 + 64, :], av[0:64, :], rb_sb[:])
                    outs.append(at)
            return outs

        def o_proj_residual_ko(w, tag, attn_tiles, bo_name, resid_tiles, rpool):
            """k-outer O-projection + bias + residual: starts on attn tile 0
            while later head-pairs are still finishing."""
            outs = []
            with tc.tile_pool(name=f"ps_{tag}", bufs=1, space=PSUM) as pp:
                pss = [pp.tile([128, TOK], F32, name=f"ps{m}", tag=f"ps{m}") for m in range(FT)]
                for k in range(FT):
                    for m in range(FT):
                        nc.tensor.matmul(pss[m][:], w(k, m * 128, (m + 1) * 128),
                                         attn_tiles[k][:], start=(k == 0), stop=(k == FT - 1))
                for m in range(FT):
                    rt = rpool.tile([128, TOK], F32, name=f"r_{tag}_{m}", tag=f"r{m}")
                    nc.vector.scalar_tensor_tensor(rt[:], pss[m][:], bias_col(bo_name, m),
                                                   resid_tiles[m][:], ADD, ADD)
                    outs.append(rt)
            return outs

        def layer_norm(r_tiles, g_name, b_name, out_dtype, out_pool, want_bf16,
                       interleave=None):
            inter_result = None
            with tc.tile_pool(name=f"lnp_{g_name}", bufs=1, space=PSUM) as lnps, \
                 tc.tile_pool(name=f"lnb_{g_name}", bufs=1, space=PSUM) as lnbc, \
                 tc.tile_pool(name=f"lns_{g_name}", bufs=1) as lnsm, \
                 tc.tile_pool(name=f"lnq_{g_name}", bufs=2) as sqp:
                s1 = lnps.tile([1, TOK], F32, tag="s1")
                s2 = lnps.tile([1, TOK], F32, tag="s2")
                for k in range(FT):
                    nc.tensor.matmul(s1[:], ones_f32[:], r_tiles[k][:],
                                     start=(k == 0), stop=(k == FT - 1))
                for k in range(FT):
                    sq = sqp.tile([128, TOK], F32, tag="sq")
                    nc.vector.tensor_mul(sq[:], r_tiles[k][:], r_tiles[k][:])
                    nc.tensor.matmul(s2[:], ones_f32[:], sq[:],
                                     start=(k == 0), stop=(k == FT - 1))
                mean = lnsm.tile([1, TOK], F32, tag="mean")
                nc.vector.tensor_scalar_mul(mean[:], s1[:], 1.0 / D)
                var = lnsm.tile([1, TOK], F32, tag="var")
                # var = s2/D - mean^2  ==  (s2 * 1/D) + (-mean*mean)
                nc.vector.scalar_tensor_tensor(var[:], mean[:], -1.0, mean[:], MULT, MULT)
                nc.vector.scalar_tensor_tensor(var[:], s2[:], 1.0 / D, var[:], MULT, ADD)
                nc.vector.tensor_scalar_add(var[:], var[:], 1e-5)
                std = lnsm.tile([1, TOK], F32, tag="std")
                nc.scalar.activation(std[:], var[:], mybir.ActivationFunctionType.Sqrt)
                # broadcasts run as bf16 matmuls (1 cycle/row vs 4 for fp32);
                # mean/rstd are small relative corrections so bf16 is plenty
                mean_bf = lnsm.tile([1, TOK], BF, tag="mean_bf")
                nc.vector.tensor_copy(mean_bf[:], mean[:])
                rstd_bf = lnsm.tile([1, TOK], BF, tag="rstd_bf")
                with nc.allow_low_precision(reason="LN rstd broadcast in bf16"):
                    nc.vector.reciprocal(rstd_bf[:], std[:])
                mean_b = lnbc.tile([128, TOK], F32, tag="meanb")
                nc.tensor.matmul(mean_b[:], ones_row_bf[:], mean_bf[:],
                                 start=True, stop=True)
                rstd_b = lnbc.tile([128, TOK], F32, tag="rstdb")
                nc.tensor.matmul(rstd_b[:], ones_row_bf[:], rstd_bf[:],
                                 start=True, stop=True)
                if interleave is not None:
                    # emit independent PE work (cross K projection) here so the
                    # tensor engine streams while the LN tail runs on DVE
                    inter_result = interleave()
                outs, outs_bf = [], []
                for k in range(FT):
                    ot = out_pool.tile([128, TOK], out_dtype, name=f"ln_{g_name}_{k}",
                                       tag=f"ln_{k}", bufs=1)
                    if unit_ln:
                        # gamma==1, beta==0: out = (r - mean) * rstd in 2 DVE ops
                        xn = sqp.tile([128, TOK], F32, tag="xn")
                        nc.vector.tensor_sub(xn[:], r_tiles[k][:], mean_b[:])
                        nc.vector.tensor_mul(ot[:], xn[:], rstd_b[:])
                    else:
                        xn = sqp.tile([128, TOK], F32, tag="xn")
                        nc.vector.tensor_sub(xn[:], r_tiles[k][:], mean_b[:])
                        nc.vector.tensor_mul(xn[:], xn[:], rstd_b[:])
                        nc.vector.tensor_scalar(ot[:], xn[:], bias_col(g_name, k),
                                                bias_col(b_name, k), MULT, ADD)
                    outs.append(ot)
                    if want_bf16:
                        ob = out_pool.tile([128, TOK], BF, name=f"lnb_{g_name}_{k}",
                                           tag=f"lnbf_{k}", bufs=1)
                        nc.scalar.activation(ob[:], ot[:], IDENT)
                        outs_bf.append(ob)
            return outs, outs_bf, inter_result

        # ================= phases 1-3: attention blocks =====================
        # One shared kv pool: cross K/V reuse self K/V buffers (same tags) —
        # self K/V are dead once self-attention completes.
        def load_w_parts(w_name, pool, parts=8):
            """Emit `parts` DMAs for a [D, cols] weight; returns accessor."""
            cols = g[w_name].shape[1]
            per = FT // parts
            tiles = []
            for pi in range(parts):
                wt = pool.tile([128, per, cols], BF, name=f"w_{w_name}_{pi}", tag=f"w{pi}")
                nc.sync.dma_start(
                    wt[:], g[w_name].rearrange("(a p) d -> p a d", p=128)[:, pi * per:(pi + 1) * per, :])
                tiles.append(wt)

            def wslice(k, c0, c1):
                return tiles[k // per][:, k % per, c0:c1]
            return wslice

        with tc.tile_pool(name="a2", bufs=FT) as a2pool:
            with tc.tile_pool(name="a1", bufs=FT) as a1pool, \
                 tc.tile_pool(name="kv", bufs=1) as kv_pool, \
                 tc.tile_pool(name="mask", bufs=1) as mask_pool:
                # phase 1: Q first (x0cb bf16 is a direct input so the
                # first matmul only waits on 2.5MB of DMA)
                with tc.tile_pool(name="x0cb", bufs=1) as xcb_pool, \
                     tc.tile_pool(name="w_sWq", bufs=1) as wq_pool:
                    x0cb_b = xcb_pool.tile([128, FT, TOK], BF, name="x0cb_b", tag="xb")
                    nc.sync.dma_start(x0cb_b[:], g['x0cb'].rearrange("(a p) t -> p a t", p=128)[:])
                    x0cb = [x0cb_b[:, k, :] for k in range(FT)]
                    wq = load_w_parts('sWq', wq_pool, parts=8)
                    emit_const_dmas()
                    maskT_t = None
                    if g['maskT'] is not None:
                        mt_b = mask_pool.tile([128, KT, TOK], BF, name="mt_b", tag="mt")
                        nc.sync.dma_start(mt_b[:], g['maskT'].rearrange("(a p) t -> p a t", p=128)[:])
                        maskT_t = [mt_b[:, k, :] for k in range(KT)]
                    q_self = proj_fm_ko(wq, x0cb, TOK, 'sbq', kv_pool, scale=0.125,
                                        tag_prefix="o_Q")
                with tc.tile_pool(name="acts_x0", bufs=1) as actp, \
                     tc.tile_pool(name="w_sWk", bufs=1) as wk_pool, \
                     tc.tile_pool(name="w_sWv", bufs=1) as wv_pool:
                    # interleave x0 k-tile and sWk part DMAs so the k-outer
                    # K projection starts after ~1.5MB of DMA
                    x0r = g['x0fm'].rearrange("(a p) t -> p a t", p=128)
                    wkr = g['sWk'].rearrange("(a p) d -> p a d", p=128)
                    x0_t, wk_tiles = [], []
                    for k in range(FT):
                        xt = actp.tile([128, S], BF, name=f"x0_{k}", tag=f"x0_{k}")
                        nc.sync.dma_start(xt[:], x0r[:, k, :])
                        x0_t.append(xt)
                        wt = wk_pool.tile([128, 1, D], BF, name=f"w_sWk_{k}", tag=f"w{k}")
                        nc.sync.dma_start(wt[:], wkr[:, k:k + 1, :])
                        wk_tiles.append(wt)
                    wk = lambda k, c0, c1: wk_tiles[k][:, 0, c0:c1]
                    wv = load_w_parts('sWv', wv_pool, parts=8)
                    k_self = proj_fm_ko(wk, x0_t, S, 'sbk', kv_pool, tag_prefix="o_K")
                    v_self = proj_tm_ko(wv, x0_t, free_bias['sbv'], kv_pool, tag_prefix="o_V")

                # phase 2: self attention + O-proj + LN1.  enc / sWo / x0c
                # DMAs are emitted before the attention body so they stream
                # during attention compute.
                with tc.tile_pool(name="acts_enc", bufs=1) as encp, \
                     tc.tile_pool(name="x0c", bufs=1) as x0c_pool, \
                     tc.tile_pool(name="r1p", bufs=1) as r1_pool:
                    encr = g['encfm'].rearrange("(a p) t -> p a t", p=128)
                    enc_t = []
                    for k in range(FT):
                        et = encp.tile([128, S], BF, name=f"enc_{k}", tag=f"enc_{k}")
                        nc.sync.dma_start(et[:], encr[:, k, :])
                        enc_t.append(et)
                    x0c_b = x0c_pool.tile([128, FT, TOK], F32, name="x0c_b", tag="x0c")
                    nc.sync.dma_start(x0c_b[:], g['x0chunk'].rearrange("(a p) t -> p a t", p=128)[:])
                    x0c_t = [x0c_b[:, k, :] for k in range(FT)]

                    with tc.tile_pool(name="w_sWo", bufs=1) as wo_pool_s:
                        swo = load_w_parts('sWo', wo_pool_s, parts=2)
                        with tc.tile_pool(name="at_s", bufs=1) as at_pool_s:
                            attn1 = attention(q_self, k_self, v_self, maskT_t, "s", at_pool_s)
                        r1 = o_proj_residual_ko(swo, "so", attn1, 'sbo', x0c_t, r1_pool)

                    with tc.tile_pool(name="w_cWk", bufs=1) as wck_pool:
                        cwk = load_w_parts('cWk', wck_pool, parts=8)

                        def inter1():
                            return proj_fm('cWk', enc_t, S, 'cbk', kv_pool,
                                           tag_prefix="o_K", w_acc=cwk)
                        a1, a1b, k_cross = layer_norm(r1, 'ln1_g', 'ln1_b', F32,
                                                      a1pool, True, interleave=inter1)
                    v_cross = proj_tm('cWv', enc_t, free_bias['cbv'], kv_pool,
                                      tag_prefix="o_V")

                # phase 3: cross attention + O-proj + LN2
                with tc.tile_pool(name="qc", bufs=1) as qc_pool, \
                     tc.tile_pool(name="maskc", bufs=1) as maskc_pool, \
                     tc.tile_pool(name="w_cWo", bufs=1) as wo_pool_c, \
                     tc.tile_pool(name="r2p", bufs=1) as r2_pool:
                    maskTc_t = None
                    if g['maskTc'] is not None:
                        mtc_b = maskc_pool.tile([128, KT, TOK], BF, name="mtc_b", tag="mtc")
                        nc.sync.dma_start(mtc_b[:], g['maskTc'].rearrange("(a p) t -> p a t", p=128)[:])
                        maskTc_t = [mtc_b[:, k, :] for k in range(KT)]
                    q_cross = proj_fm('cWq', a1b, TOK, 'cbq', qc_pool, scale=0.125)
                    cwo = load_w_parts('cWo', wo_pool_c, parts=2)
                    with tc.tile_pool(name="at_c", bufs=1) as at_pool_c:
                        attn2 = attention(q_cross, k_cross, v_cross, maskTc_t, "c", at_pool_c)
                        r2 = o_proj_residual_ko(cwo, "co", attn2, 'cbo', a1, r2_pool)
                    a2, a2b, _ = layer_norm(r2, 'ln2_g', 'ln2_b', F32, a2pool, True)

            # ================= phase 4: FFN + LN3 (chunked weights) =========
            with tc.tile_pool(name="r3p", bufs=1) as r3_pool:
                r3 = [r3_pool.tile([128, TOK], F32, name=f"r_ffn_{m}", tag=f"r{m}")
                      for m in range(FT)]
                with tc.tile_pool(name="w_f1", bufs=2) as wp1, \
                     tc.tile_pool(name="w_f2", bufs=2) as wp2, \
                     tc.tile_pool(name="hid", bufs=2) as hpool, \
                     tc.tile_pool(name="ps_f1", bufs=3, space=PSUM) as pp1, \
                     tc.tile_pool(name="ps_f2", bufs=3, space=PSUM) as pp2:
                    f1r = g['fW1'].rearrange("(a p) d -> p a d", p=128)
                    f2r = g['fW2'].rearrange("(a p) d -> p a d", p=128)
                    for ci in range(NCH):
                        w1c = wp1.tile([128, FT, CHT * 128], BF, tag="w1")
                        nc.sync.dma_start(w1c[:], f1r[:, :, ci * CHT * 128:(ci + 1) * CHT * 128])
                        w2c = wp2.tile([128, CHT, D], BF, tag="w2")
                        nc.sync.dma_start(w2c[:], f2r[:, ci * CHT:(ci + 1) * CHT, :])
                        hbuf = hpool.tile([128, CHT, TOK], BF, tag="h")
                        for mh in range(CHT):
                            ps = pp1.tile([128, TOK], F32, tag="ps")
                            for k in range(FT):
                                nc.tensor.matmul(ps[:], w1c[:, k, mh * 128:(mh + 1) * 128],
                                                 a2b[k][:], start=(k == 0), stop=(k == FT - 1))
                            nc.scalar.activation(hbuf[:, mh, :], ps[:],
                                                 mybir.ActivationFunctionType.Relu,
                                                 bias=fb1_sb[:, ci * CHT + mh:ci * CHT + mh + 1])
                        for m in range(FT):
                            ps = pp2.tile([128, TOK], F32, tag="ps")
                            for kh in range(CHT):
                                nc.tensor.matmul(ps[:], w2c[:, kh, m * 128:(m + 1) * 128],
                                                 hbuf[:, kh, :], start=(kh == 0), stop=(kh == CHT - 1))
                            if ci == 0:
                                nc.vector.scalar_tensor_tensor(r3[m][:], ps[:], bias_col('fb2', m),
                                                               a2[m][:], ADD, ADD)
                            else:
                                nc.vector.tensor_add(r3[m][:], r3[m][:], ps[:])
                y, _, _ = layer_norm(r3, 'ln3_g', 'ln3_b', BF, ypool, False)

            # Wout ring prefetch: queued behind the FFN weight DMAs, streams
            # during FFN/LN3 compute.
            for n in range(RING):
                emit_wo_load(n)

        # ================= phase 5: vocab projection ========================
        with tc.tile_pool(name="vout", bufs=4) as vos, \
             tc.tile_pool(name="vps", bufs=4, space=PSUM) as vps:
            for n in range(VN):
                wt = wo_tiles.pop(n)
                for m in range(2):
                    ps = vps.tile([128, VC], F32, tag="ps")
                    for k in range(FT):
                        nc.tensor.matmul(ps[:], y[k][:, m * 128:(m + 1) * 128],
                                         wt[:, k, :], start=(k == 0), stop=(k == FT - 1))
                    ot = vos.tile([128, VC], BF, tag="vo")
                    copy_out(ot[:], ps[:], n * 2 + m)
                    nc.gpsimd.dma_start(g['out'][m * 128:(m + 1) * 128, n * VC:(n + 1) * VC], ot[:])
                if n + RING < VN:
                    emit_wo_load(n + RING)


def host_prep(inputs):
    x0 = np.asarray(inputs['dec_input'], np.float32) + positional_encoding(S, D)[None]
    enc = np.asarray(inputs['enc_input'], np.float32)
    mask_self = np.asarray(inputs['masked_attention_mask'], np.float32)[0, 0]
    mask_cross = np.asarray(inputs['cross_attention_mask'], np.float32)[0, 0]
    self_adds = bool(np.any(mask_self != 0.0))
    cross_adds = bool(np.any(mask_cross != 0.0))
    li = L - 1
    Wl = {}
    for p in ['sWq', 'sWk', 'sWv', 'sWo', 'cWq', 'cWk', 'cWv', 'cWo', 'fW1', 'fW2']:
        Wl[p] = np.ascontiguousarray(np.asarray(inputs[p], np.float32)[li]).astype(BF16)
    bv = {}
    for p in ['sbq', 'sbk', 'sbv', 'sbo', 'cbq', 'cbk', 'cbv', 'cbo',
              'ln1_g', 'ln1_b', 'ln2_g', 'ln2_b', 'ln3_g', 'ln3_b', 'fb1', 'fb2']:
        bv[p] = np.asarray(inputs[p], np.float32)[li]
    Wout_bf = np.ascontiguousarray(np.asarray(inputs['Wout'], np.float32)).astype(BF16)
    bout = np.asarray(inputs['bout'], np.float32)
    ident = np.eye(128, dtype=BF16)

    def pp(v):  # [1024] -> [128, 8] partition-major
        return np.ascontiguousarray(v.reshape(-1, 128).T)

    bias_cols = []
    for name in BIAS_NAMES:
        src = {'sbq': bv['sbq'] * 0.125, 'cbq': bv['cbq'] * 0.125}.get(name, bv.get(name))
        bias_cols.append(pp(src))
    biases_pp = np.ascontiguousarray(np.concatenate(bias_cols, axis=1), np.float32)
    fb1_pp = np.ascontiguousarray(bv['fb1'].reshape(HT, 128).T, np.float32)

    in_maps = []
    for core in range(NC):
        b, c = core // 4, core % 4
        q0 = c * TOK
        x0c = np.ascontiguousarray(x0[b, q0:q0 + TOK].T)
        m = {
            'x0fm': np.ascontiguousarray(x0[b].T).astype(BF16),
            'encfm': np.ascontiguousarray(enc[b].T).astype(BF16),
            'x0chunk': np.ascontiguousarray(x0c, np.float32),
            'x0cb': x0c.astype(BF16),
            'biases': biases_pp, 'fb1': fb1_pp, 'ident': ident,
            'sbv_row': np.ascontiguousarray(bv['sbv'][None, :], np.float32),
            'cbv_row': np.ascontiguousarray(bv['cbv'][None, :], np.float32),
            'Wout': Wout_bf,
        }
        m.update(Wl)
        if self_adds:
            m['maskT'] = np.ascontiguousarray(mask_self[q0:q0 + TOK, :].T).astype(BF16)
        if cross_adds:
            m['maskTc'] = np.ascontiguousarray(mask_cross[q0:q0 + TOK, :].T).astype(BF16)
        in_maps.append(m)
    zero_free = not (np.any(bv['sbv']) or np.any(bv['cbv']))
    unit_ln = all(np.all(bv[f'ln{i}_g'] == 1.0) and not np.any(bv[f'ln{i}_b'])
                  for i in (1, 2, 3))
    return in_maps, self_adds, cross_adds, zero_free, unit_ln, bout


_CACHE = {}


def _get_program(self_adds, cross_adds, zero_free, unit_ln):
    key = (self_adds, cross_adds, zero_free, unit_ln)
    if key not in _CACHE:
        _CACHE[key] = build_program(self_adds, cross_adds, zero_free, unit_ln)
    return _CACHE[key]


def kernel(**inputs):
    in_maps, self_adds, cross_adds, zero_free, unit_ln, bout = host_prep(inputs)
    nc = _get_program(self_adds, cross_adds, zero_free, unit_ln)
    res = run_bass_kernel_spmd(nc, in_maps, core_ids=list(range(NC)))
    shards = [np.asarray(res.results[r]["out"], dtype=np.float32) for r in range(NC)]
    full = np.concatenate(shards, axis=0)           # [2048, V]
    if np.any(bout):
        full = full + bout[None, :]
    return np.ascontiguousarray(full.reshape(B, S, V), np.float32)


# revision 40
# speedup vs baseline: 1.3909x; 1.0127x over previous
"""Trainium2 Bass kernel for nn_Decoder_3539053052044.

Structure (v2 — no collectives):
- The reference decoder has a preserved bug: every layer consumes the ORIGINAL
  x0, so only the LAST layer's output survives. We compute layer L-1 only.
- Sequence-parallel: 8 cores x 256 tokens (core r -> batch r//4, chunk r%4).
  Each core computes the full last layer for its 256 tokens (K/V projections
  for its whole batch are computed locally), then projects its OWN 256 tokens
  against the FULL vocab (full Wout streamed from HBM in 500-col chunks
  through a 9-deep SBUF ring) — zero cross-core communication.
- Activations are feature-major [D on partitions, tokens free] so every linear
  layer uses the stored [D_in, D_out] weights directly as lhsT.
- Softmax is max-free (scores are O(1) for this model; exp(-1e9)=0 handles
  masking) and computed directly transposed, scoresT[k,q]. The additive mask
  is folded into the scores PSUM accumulation group via an identity-weight
  matmul (PE), so exp reads PSUM directly — no DVE mask-add, no SBUF staging.
  V carries a ones column per head ([128, H*65]) so the AV matmul's 65th
  output row is the softmax denominator for free.
- LayerNorm stat broadcasts run as bf16 matmuls (1 cycle/row vs 4 for fp32);
  k-outer projections (8 psum banks accumulating across the contraction) start
  as soon as the first weight k-tile arrives, with per-k-tile interleaved DMAs.
- FFN weights are streamed in 8 hidden-chunks (1MB each, double buffered) with
  the fW2 contraction accumulated into SBUF, freeing SBUF for the Wout ring.
- Output is written bf16 ([256, 32000] per core) and converted to fp32 on the
  host; vocab bias (all-zero here) would be added host-side.
- bf16 matmul inputs, fp32 accumulation.
"""

import numpy as np
import ml_dtypes

import concourse.bass as bass
import concourse.bacc as bacc
import concourse.tile as tile
from concourse import mybir
from concourse.bass_utils import run_bass_kernel_spmd
from concourse.vector_clock import ScopedClock, VectorClock

BF16 = ml_dtypes.bfloat16
F32 = mybir.dt.float32
F32R = mybir.dt.float32r
BF = mybir.dt.bfloat16
PSUM = bass.MemorySpace.PSUM

B, S, D, H, L, V, DF = 2, 1024, 1024, 16, 4, 32000, 4096
DH = D // H              # 64
NC = 8                   # cores
TOK = B * S // NC        # 256 tokens per core
KT = S // 128            # 8 k tiles
FT = D // 128            # 8 feature tiles
HT = DF // 128           # 32 hidden tiles
NCH = 8                  # FFN hidden chunks
CHT = HT // NCH          # 4 hidden tiles per chunk
VC = 500                 # vocab cols per chunk
VN = V // VC             # 64 vocab chunks
RING = 9                 # Wout ring depth
ADD = mybir.AluOpType.add
MULT = mybir.AluOpType.mult
IDENT = mybir.ActivationFunctionType.Identity
ADD_OP = mybir.AluOpType.add

_PATCHED = False


def _patch_tile_drain():
    """This neuronxcc build rejects a Drain carrying >1 sem wait. Split the
    Tile tail drain into one Drain per busy proc, each with a single wait."""
    global _PATCHED
    if _PATCHED:
        return
    _PATCHED = True

    def _drain_and_barrier_split(self, tick_clock, wait_clock):
        gc = tick_clock.global_clock
        n = len(gc)
        for p in range(n):
            if gc[p] > 0:
                vc = VectorClock([gc[q] if q == p else 0 for q in range(n)])
                d = self.nc.sync.drain()
                wait_clock.add_sem_waits(d.ins, ScopedClock({None: vc}))
        self.nc.sync.drain()
        self.nc.all_engine_barrier()
        assert self.sems is not None
        popped = self.nc._tile_sem_poison_stack.pop()
        assert popped is self._sem_poison
        self.nc.clear_and_free_semaphores(list(self.sems.allocated().values()))
        self.nc.all_engine_barrier()

    tile.TileContext._drain_and_barrier = _drain_and_barrier_split


def positional_encoding(seq_len, d_model, n=10000.0):
    i = np.arange(seq_len, dtype=np.float32)[:, None]
    d = np.arange(d_model)
    denom = np.power(n, (2 * (d // 2)).astype(np.float32) / d_model)
    ang = i / denom
    return np.where(d % 2 == 0, np.sin(ang), np.cos(ang)).astype(np.float32)


BIAS_NAMES = ['sbk', 'sbq', 'sbo', 'cbk', 'cbq', 'cbo', 'fb2',
              'ln1_g', 'ln1_b', 'ln2_g', 'ln2_b', 'ln3_g', 'ln3_b']


def build_program(self_mask_adds: bool, cross_mask_adds: bool, zero_free_biases: bool = False,
                  unit_ln: bool = False):
    _patch_tile_drain()
    nc = bacc.Bacc()

    g = {}  # dram handles
    g['x0fm'] = nc.declare_dram_parameter("x0fm", [D, S], BF, isOutput=False)
    g['encfm'] = nc.declare_dram_parameter("encfm", [D, S], BF, isOutput=False)
    g['x0chunk'] = nc.declare_dram_parameter("x0chunk", [D, TOK], F32, isOutput=False)
    g['x0cb'] = nc.declare_dram_parameter("x0cb", [D, TOK], BF, isOutput=False)
    for w in ['sWq', 'sWk', 'sWv', 'sWo', 'cWq', 'cWk', 'cWv', 'cWo']:
        g[w] = nc.declare_dram_parameter(w, [D, D], BF, isOutput=False)
    g['fW1'] = nc.declare_dram_parameter("fW1", [D, DF], BF, isOutput=False)
    g['fW2'] = nc.declare_dram_parameter("fW2", [DF, D], BF, isOutput=False)
    g['Wout'] = nc.declare_dram_parameter("Wout", [D, V], BF, isOutput=False)
    g['biases'] = nc.declare_dram_parameter("biases", [128, 8 * len(BIAS_NAMES)], F32, isOutput=False)
    g['fb1'] = nc.declare_dram_parameter("fb1", [128, HT], F32, isOutput=False)
    g['ident'] = nc.declare_dram_parameter("ident", [128, 128], BF, isOutput=False)
    g['sbv_row'] = nc.declare_dram_parameter("sbv_row", [1, D], F32, isOutput=False)
    g['cbv_row'] = nc.declare_dram_parameter("cbv_row", [1, D], F32, isOutput=False)
    g['maskT'] = nc.declare_dram_parameter("maskT", [S, TOK], BF, isOutput=False) if self_mask_adds else None
    g['maskTc'] = nc.declare_dram_parameter("maskTc", [S, TOK], BF, isOutput=False) if cross_mask_adds else None
    g['out'] = nc.declare_dram_parameter("out", [TOK, V], BF, isOutput=True)

    with tile.TileContext(nc) as tc:
        _emit(nc, tc, g, zero_free_biases, unit_ln)
    nc.compile()
    return nc


def _emit(nc, tc, g, zero_free_biases, unit_ln):
    from contextlib import ExitStack
    ctx = ExitStack()
    with ctx:
        # ---------- whole-kernel constants / small tensors ------------------
        const = ctx.enter_context(tc.tile_pool(name="const", bufs=1))
        ones_bf = const.tile([128, 1], BF, name="ones_bf", tag="c0")
        nc.gpsimd.memset(ones_bf[:], 1.0)
        ones_f32 = const.tile([128, 1], F32, name="ones_f32", tag="c1")
        nc.gpsimd.memset(ones_f32[:], 1.0)
        ones_row = const.tile([1, 128], F32, name="ones_row", tag="c2")
        nc.gpsimd.memset(ones_row[:], 1.0)
        ones_row_bf = const.tile([1, 128], BF, name="ones_row_bf", tag="c6")
        nc.gpsimd.memset(ones_row_bf[:], 1.0)
        # const DMAs are emitted in phase 1 after the x0cb/sWq loads (they are
        # not needed until the first bias add) so the first matmul starts early
        bias_sb = const.tile([128, 8 * len(BIAS_NAMES)], F32, name="bias_sb", tag="c3")
        fb1_sb = const.tile([128, HT], F32, name="fb1_sb", tag="c4")
        ident_sb = const.tile([128, 128], BF, name="ident_sb", tag="c5")

        def emit_const_dmas():
            nc.sync.dma_start(bias_sb[:], g['biases'][:])
            nc.sync.dma_start(fb1_sb[:], g['fb1'][:])
            nc.sync.dma_start(ident_sb[:], g['ident'][:])

        def bias_col(name, f):
            i = BIAS_NAMES.index(name)
            return bias_sb[:, i * 8 + f:i * 8 + f + 1]

        # free-axis bias broadcast tiles [128, D] for sbv / cbv (skipped when
        # the host observed all-zero free-axis biases)
        free_bias = {'sbv': None, 'cbv': None}
        if not zero_free_biases:
            with tc.tile_pool(name="bbc_ps", bufs=1, space=PSUM) as bps, \
                 tc.tile_pool(name="bbc_row", bufs=2) as brow:
                for bi, bname in enumerate(['sbv', 'cbv']):
                    t = const.tile([128, D], F32, name=f"{bname}_b", tag=f"fb{bi}")
                    rsb = brow.tile([1, D], F32, tag="row")
                    nc.sync.dma_start(rsb[:], g[f'{bname}_row'][:])
                    for half in range(2):
                        ps = bps.tile([128, 512], F32, tag="bc")
                        nc.tensor.matmul(ps[:], ones_row[:],
                                         rsb[0:1, half * 512:(half + 1) * 512],
                                         start=True, stop=True)
                        nc.vector.tensor_copy(t[:, half * 512:(half + 1) * 512], ps[:])
                    free_bias[bname] = t

        # Wout ring: reserved at the bottom of the SBUF stack for the whole
        # program so the prefetch DMAs never anti-depend on phase pools.
        wo_pool = ctx.enter_context(tc.tile_pool(name="wo_ring", bufs=RING))
        wo_tiles = {}

        def emit_wo_load(n):
            t = wo_pool.tile([128, FT, VC], BF, name=f"wo_{n}", tag="wo")
            nc.sync.dma_start(
                t[:], g['Wout'].rearrange("(a p) v -> p a v", p=128)[:, :, n * VC:(n + 1) * VC])
            wo_tiles[n] = t

        def copy_out(ot, ps, idx):
            """psum->sbuf copy alternating DVE/ACT to balance engines"""
            if idx % 2 == 0:
                nc.vector.tensor_copy(ot, ps)
            else:
                nc.scalar.activation(ot, ps, IDENT)

        def load_w(w_name, pool, kt_n, cols, parts=1):
            """Load [kt_n*128, cols] weight as `parts` batched tiles.
            Returns accessor: wslice(k, c0, c1) -> [128, c1-c0] lhsT/rhs AP."""
            per = kt_n // parts
            tiles = []
            for pi in range(parts):
                wt = pool.tile([128, per, cols], BF, name=f"w_{w_name}_{pi}", tag=f"w{pi}")
                nc.sync.dma_start(
                    wt[:], g[w_name].rearrange("(a p) d -> p a d", p=128)[:, pi * per:(pi + 1) * per, :])
                tiles.append(wt)

            def wslice(k, c0, c1):
                return tiles[k // per][:, k % per, c0:c1]
            return wslice

        # LN outputs: a1 lives through phase 3; a2 through FFN; y to the end.
        ypool = ctx.enter_context(tc.tile_pool(name="y", bufs=FT))

        # ---------- helpers -------------------------------------------------
        def proj_fm(w_name, act_tiles, n_tok, bias_name, out_pool, scale=None,
                    tag_prefix=None, w_acc=None):
            """feature-major out tiles [FT x [128, n_tok]] bf16 = W.T @ act + b"""
            outs = []
            tp = tag_prefix or f"o_{w_name}"
            nsub = (n_tok + 511) // 512
            from contextlib import nullcontext
            wctx = nullcontext() if w_acc else tc.tile_pool(name=f"w_{w_name}", bufs=1)
            with wctx as wp, \
                 tc.tile_pool(name=f"ps_{w_name}", bufs=4, space=PSUM) as pp:
                w = w_acc or load_w(w_name, wp, FT, D, parts=2)
                for m in range(FT):
                    ot = out_pool.tile([128, n_tok], BF, name=f"o_{w_name}_{m}",
                                       tag=f"{tp}_{m}", bufs=1)
                    for ns in range(nsub):
                        c0, c1 = ns * 512, min((ns + 1) * 512, n_tok)
                        ps = pp.tile([128, c1 - c0], F32, tag="ps")
                        for k in range(FT):
                            nc.tensor.matmul(ps[:], w(k, m * 128, (m + 1) * 128),
                                             act_tiles[k][:, c0:c1],
                                             start=(k == 0), stop=(k == FT - 1))
                        if scale is not None:
                            nc.vector.tensor_scalar(ot[:, c0:c1], ps[:], scale,
                                                    bias_col(bias_name, m), MULT, ADD)
                        elif (m + ns) % 2 == 0:
                            nc.vector.tensor_scalar(ot[:, c0:c1], ps[:], 1.0,
                                                    bias_col(bias_name, m), MULT, ADD)
                        else:
                            nc.scalar.activation(ot[:, c0:c1], ps[:], IDENT,
                                                 bias=bias_col(bias_name, m))
                    outs.append(ot)
            return outs

        def proj_fm_ko(w, act_tiles, n_tok, bias_name, out_pool, scale=None,
                       tag_prefix="o"):
            """k-outer feature-major projection: all FT output psums accumulate
            simultaneously (8 banks) so compute starts as soon as the first
            weight k-tile and activation k-tile arrive."""
            outs = []
            nsub = (n_tok + 511) // 512
            with tc.tile_pool(name=f"ps_{tag_prefix}", bufs=1, space=PSUM) as pp:
                for m in range(FT):
                    outs.append(out_pool.tile([128, n_tok], BF, name=f"{tag_prefix}{m}",
                                              tag=f"{tag_prefix}_{m}", bufs=1))
                for ns in range(nsub):
                    c0, c1 = ns * 512, min((ns + 1) * 512, n_tok)
                    pss = [pp.tile([128, c1 - c0], F32, name=f"ps{m}", tag=f"ps{m}") for m in range(FT)]
                    for k in range(FT):
                        for m in range(FT):
                            nc.tensor.matmul(pss[m][:], w(k, m * 128, (m + 1) * 128),
                                             act_tiles[k][:, c0:c1],
                                             start=(k == 0), stop=(k == FT - 1))
                    for m in range(FT):
                        if scale is not None:
                            nc.vector.tensor_scalar(outs[m][:, c0:c1], pss[m][:], scale,
                                                    bias_col(bias_name, m), MULT, ADD)
                        elif m % 2 == 0:
                            nc.vector.tensor_scalar(outs[m][:, c0:c1], pss[m][:], 1.0,
                                                    bias_col(bias_name, m), MULT, ADD)
                        else:
                            nc.scalar.activation(outs[m][:, c0:c1], pss[m][:], IDENT,
                                                 bias=bias_col(bias_name, m))
            return outs

        def proj_tm_ko(w, act_tiles, bias_bcast, out_pool, tag_prefix="oV"):
            """k-outer token-major V projection with the ones-column trick."""
            outs = []
            with tc.tile_pool(name=f"ps_{tag_prefix}", bufs=1, space=PSUM) as pp:
                for m in range(KT):
                    ot = out_pool.tile([128, H * 65], BF, name=f"{tag_prefix}{m}",
                                       tag=f"{tag_prefix}_{m}", bufs=1)
                    ones_cols = ot.rearrange("p (h c) -> p h c", c=65)[:, :, 64:65]
                    nc.gpsimd.memset(ones_cols, 1.0)
                    outs.append(ot)
                for ns in range(2):
                    c0, c1 = ns * 512, (ns + 1) * 512
                    pss = [pp.tile([128, 512], F32, name=f"ps{m}", tag=f"ps{m}") for m in range(KT)]
                    for k in range(FT):
                        for m in range(KT):
                            nc.tensor.matmul(pss[m][:], act_tiles[k][:, m * 128:(m + 1) * 128],
                                             w(k, c0, c1), start=(k == 0), stop=(k == FT - 1))
                    for m in range(KT):
                        dst = outs[m][:, ns * 8 * 65:(ns * 8 + 8) * 65].rearrange(
                            "p (h c) -> p h c", c=65)[:, :, 0:64]
                        psv = pss[m].rearrange("p (h c) -> p h c", c=64)
                        if bias_bcast is None:
                            copy_out(dst, psv, m + ns)
                        else:
                            bbv = bias_bcast[:, c0:c1].rearrange("p (h c) -> p h c", c=64)
                            nc.vector.scalar_tensor_tensor(dst, psv, 1.0, bbv, MULT, ADD)
            return outs

        def proj_tm(w_name, act_tiles, bias_bcast, out_pool, tag_prefix=None):
            """token-major V tiles [KT x [128, H*65]] bf16 = act.T @ W + b, with
            a ones column appended after each head's 64 dims so the AV matmul's
            65th output row is the softmax denominator for free."""
            outs = []
            tp = tag_prefix or f"o_{w_name}"
            with tc.tile_pool(name=f"w_{w_name}", bufs=1) as wp, \
                 tc.tile_pool(name=f"ps_{w_name}", bufs=4, space=PSUM) as pp:
                w = load_w(w_name, wp, FT, D, parts=2)
                for m in range(KT):
                    ot = out_pool.tile([128, H * 65], BF, name=f"o_{w_name}_{m}",
                                       tag=f"{tp}_{m}", bufs=1)
                    ones_cols = ot.rearrange("p (h c) -> p h c", c=65)[:, :, 64:65]
                    nc.gpsimd.memset(ones_cols, 1.0)
                    for ns in range(2):
                        c0, c1 = ns * 512, (ns + 1) * 512
                        ps = pp.tile([128, 512], F32, tag="ps")
                        for k in range(FT):
                            nc.tensor.matmul(ps[:], act_tiles[k][:, m * 128:(m + 1) * 128],
                                             w(k, c0, c1),
                                             start=(k == 0), stop=(k == FT - 1))
                        dst = ot[:, ns * 8 * 65:(ns * 8 + 8) * 65].rearrange(
                            "p (h c) -> p h c", c=65)[:, :, 0:64]
                        psv = ps.rearrange("p (h c) -> p h c", c=64)
                        if bias_bcast is None:
                            copy_out(dst, psv, m * 2 + ns)
                        else:
                            bbv = bias_bcast[:, c0:c1].rearrange("p (h c) -> p h c", c=64)
                            nc.vector.scalar_tensor_tensor(dst, psv, 1.0, bbv, MULT, ADD)
                    outs.append(ot)
            return outs

        def attention(q_pairs, k_tiles, v_tiles, mask_tiles, tag, out_pool):
            """q_pairs feature-major [FT x [128,TOK]] bf16; k_tiles [FT x [128,S]];
            v_tiles token-major [KT x [128,H*65]]; mask_tiles bf16 doubled
            [KT x [128,2*TOK]] or None. Two heads of a feature pair share one
            2-bank scoresT psum tile (one matmul group per bank). The additive
            mask joins each head's accumulation group as an identity-weight
            matmul, so exp reads PSUM directly."""
            outs = []
            with tc.tile_pool(name=f"exp_{tag}", bufs=4) as epool, \
                 tc.tile_pool(name=f"asm_{tag}", bufs=4) as spool, \
                 tc.tile_pool(name=f"sT_{tag}", bufs=3, space=PSUM) as sps, \
                 tc.tile_pool(name=f"av_{tag}", bufs=2, space=PSUM) as avs:
                for hp in range(FT):
                    at = out_pool.tile([128, TOK], BF, name=f"at_{tag}_{hp}",
                                       tag=f"at_{hp}", bufs=1)
                    av0 = avs.tile([65, TOK], F32, tag="av")
                    av1 = avs.tile([65, TOK], F32, tag="av")
                    for kt in range(KT):
                        # two heads' scoresT in one 2-bank psum tile (one matmul
                        # group per bank); mask folded into each group via an
                        # identity-weight matmul
                        sTp = sps.tile([128, 4 * TOK], F32, tag="sT")
                        for hh in range(2):
                            po = hh * 64
                            nc.tensor.matmul(
                                sTp[:, hh * 2 * TOK:hh * 2 * TOK + TOK],
                                k_tiles[hp][po:po + 64, kt * 128:(kt + 1) * 128],
                                q_pairs[hp][po:po + 64, :], start=True,
                                stop=(mask_tiles is None))
                            if mask_tiles is not None:
                                nc.tensor.matmul(
                                    sTp[:, hh * 2 * TOK:hh * 2 * TOK + TOK],
                                    ident_sb[:], mask_tiles[kt][:],
                                    start=False, stop=True)
                        sview = sTp.rearrange("p (b c) -> p b c", c=2 * TOK)[:, :, 0:TOK]
                        ex = epool.tile([128, 2 * TOK], BF, tag="exp")
                        exv = ex.rearrange("p (b c) -> p b c", c=TOK)
                        nc.scalar.activation(exv, sview, mybir.ActivationFunctionType.Exp)
                        for hh, av in ((0, av0), (1, av1)):
                            h = 2 * hp + hh
                            nc.tensor.matmul(av[:],
                                             v_tiles[kt][:, h * 65:(h + 1) * 65],
                                             ex[:, hh * TOK:(hh + 1) * TOK],
                                             start=(kt == 0), stop=(kt == KT - 1))
                    for hh, av in ((0, av0), (1, av1)):
                        rec = spool.tile([1, TOK], BF, tag="recip")
                        with nc.allow_low_precision(reason="softmax 1/sum broadcast in bf16"):
                            nc.vector.reciprocal(rec[:], av[64:65, :])
                        rb = sps.tile([64, TOK], F32, tag="sT")
                        nc.tensor.matmul(rb[:], ones_row_bf[0:1, 0:64],
                                         rec[:], start=True, stop=True)
                        rb_sb = spool.tile([64, TOK], F32, tag="rb_sb")
                        nc.vector.tensor_copy(rb_sb[:], rb[:])
                        nc.vector.tensor_mul(at[hh * 64:hh * 64 + 64, :], av[0:64, :], rb_sb[:])
                    outs.append(at)
            return outs

        def o_proj_residual_ko(w, tag, attn_tiles, bo_name, resid_tiles, rpool):
            """k-outer O-projection + bias + residual: starts on attn tile 0
            while later head-pairs are still finishing."""
            outs = []
            with tc.tile_pool(name=f"ps_{tag}", bufs=1, space=PSUM) as pp:
                pss = [pp.tile([128, TOK], F32, name=f"ps{m}", tag=f"ps{m}") for m in range(FT)]
                for k in range(FT):
                    for m in range(FT):
                        nc.tensor.matmul(pss[m][:], w(k, m * 128, (m + 1) * 128),
                                         attn_tiles[k][:], start=(k == 0), stop=(k == FT - 1))
                for m in range(FT):
                    rt = rpool.tile([128, TOK], F32, name=f"r_{tag}_{m}", tag=f"r{m}")
                    nc.vector.scalar_tensor_tensor(rt[:], pss[m][:], bias_col(bo_name, m),
                                                   resid_tiles[m][:], ADD, ADD)
                    outs.append(rt)
            return outs

        def layer_norm(r_tiles, g_name, b_name, out_dtype, out_pool, want_bf16,
                       interleave=None):
            inter_result = None
            with tc.tile_pool(name=f"lnp_{g_name}", bufs=1, space=PSUM) as lnps, \
                 tc.tile_pool(name=f"lnb_{g_name}", bufs=1, space=PSUM) as lnbc, \
                 tc.tile_pool(name=f"lns_{g_name}", bufs=1) as lnsm, \
                 tc.tile_pool(name=f"lnq_{g_name}", bufs=1) as sqp:
                s1 = lnps.tile([1, TOK], F32, tag="s1")
                s2 = lnps.tile([1, TOK], F32, tag="s2")
                for k in range(FT):
                    nc.tensor.matmul(s1[:], ones_f32[:], r_tiles[k][:],
                                     start=(k == 0), stop=(k == FT - 1))
                for k in range(FT):
                    sq = sqp.tile([128, TOK], F32, tag="sq")
                    nc.vector.tensor_mul(sq[:], r_tiles[k][:], r_tiles[k][:])
                    nc.tensor.matmul(s2[:], ones_f32[:], sq[:],
                                     start=(k == 0), stop=(k == FT - 1))
                mean = lnsm.tile([1, TOK], F32, tag="mean")
                nc.vector.tensor_scalar_mul(mean[:], s1[:], 1.0 / D)
                var = lnsm.tile([1, TOK], F32, tag="var")
                # var = s2/D - mean^2  ==  (s2 * 1/D) + (-mean*mean)
                nc.vector.scalar_tensor_tensor(var[:], mean[:], -1.0, mean[:], MULT, MULT)
                nc.vector.scalar_tensor_tensor(var[:], s2[:], 1.0 / D, var[:], MULT, ADD)
                nc.vector.tensor_scalar_add(var[:], var[:], 1e-5)
                std = lnsm.tile([1, TOK], F32, tag="std")
                nc.scalar.activation(std[:], var[:], mybir.ActivationFunctionType.Sqrt)
                # broadcasts run as bf16 matmuls (1 cycle/row vs 4 for fp32);
                # mean/rstd are small relative corrections so bf16 is plenty
                mean_bf = lnsm.tile([1, TOK], BF, tag="mean_bf")
                nc.vector.tensor_copy(mean_bf[:], mean[:])
                rstd_bf = lnsm.tile([1, TOK], BF, tag="rstd_bf")
                with nc.allow_low_precision(reason="LN rstd broadcast in bf16"):
                    nc.vector.reciprocal(rstd_bf[:], std[:])
                mean_b = lnbc.tile([128, TOK], F32, tag="meanb")
                nc.tensor.matmul(mean_b[:], ones_row_bf[:], mean_bf[:],
                                 start=True, stop=True)
                rstd_b = lnbc.tile([128, TOK], F32, tag="rstdb")
                nc.tensor.matmul(rstd_b[:], ones_row_bf[:], rstd_bf[:],
                                 start=True, stop=True)
                if interleave is not None:
                    # emit independent PE work (cross K projection) here so the
                    # tensor engine streams while the LN tail runs on DVE
                    inter_result = interleave()
                outs, outs_bf = [], []
                for k in range(FT):
                    ot = out_pool.tile([128, TOK], out_dtype, name=f"ln_{g_name}_{k}",
                                       tag=f"ln_{k}", bufs=1)
                    if unit_ln:
                        # gamma==1, beta==0: out = (r - mean) * rstd in 2 DVE ops
                        xn = sqp.tile([128, TOK], F32, tag="xn")
                        nc.vector.tensor_sub(xn[:], r_tiles[k][:], mean_b[:])
                        nc.vector.tensor_mul(ot[:], xn[:], rstd_b[:])
                    else:
                        xn = sqp.tile([128, TOK], F32, tag="xn")
                        nc.vector.tensor_sub(xn[:], r_tiles[k][:], mean_b[:])
                        nc.vector.tensor_mul(xn[:], xn[:], rstd_b[:])
                        nc.vector.tensor_scalar(ot[:], xn[:], bias_col(g_name, k),
                                                bias_col(b_name, k), MULT, ADD)
                    outs.append(ot)
                    if want_bf16:
                        ob = out_pool.tile([128, TOK], BF, name=f"lnb_{g_name}_{k}",
                                           tag=f"lnbf_{k}", bufs=1)
                        nc.scalar.activation(ob[:], ot[:], IDENT)
                        outs_bf.append(ob)
            return outs, outs_bf, inter_result

        # ================= phases 1-3: attention blocks =====================
        # One shared kv pool: cross K/V reuse self K/V buffers (same tags) —
        # self K/V are dead once self-attention completes.
        def load_w_parts(w_name, pool, parts=8):
            """Emit `parts` DMAs for a [D, cols] weight; returns accessor."""
            cols = g[w_name].shape[1]
            per = FT // parts
            tiles = []
            for pi in range(parts):
                wt = pool.tile([128, per, cols], BF, name=f"w_{w_name}_{pi}", tag=f"w{pi}")
                nc.sync.dma_start(
                    wt[:], g[w_name].rearrange("(a p) d -> p a d", p=128)[:, pi * per:(pi + 1) * per, :])
                tiles.append(wt)

            def wslice(k, c0, c1):
                return tiles[k // per][:, k % per, c0:c1]
            return wslice

        with tc.tile_pool(name="a2", bufs=FT) as a2pool:
            with tc.tile_pool(name="a1", bufs=FT) as a1pool, \
                 tc.tile_pool(name="kv", bufs=1) as kv_pool, \
                 tc.tile_pool(name="mask", bufs=1) as mask_pool:
                # phase 1: Q first (x0cb bf16 is a direct input so the
                # first matmul only waits on 2.5MB of DMA)
                with tc.tile_pool(name="x0cb", bufs=1) as xcb_pool, \
                     tc.tile_pool(name="w_sWq", bufs=1) as wq_pool:
                    x0cb_b = xcb_pool.tile([128, FT, TOK], BF, name="x0cb_b", tag="xb")
                    nc.sync.dma_start(x0cb_b[:], g['x0cb'].rearrange("(a p) t -> p a t", p=128)[:])
                    x0cb = [x0cb_b[:, k, :] for k in range(FT)]
                    wq = load_w_parts('sWq', wq_pool, parts=8)
                    emit_const_dmas()
                    maskT_t = None
                    if g['maskT'] is not None:
                        mt_b = mask_pool.tile([128, KT, TOK], BF, name="mt_b", tag="mt")
                        nc.sync.dma_start(mt_b[:], g['maskT'].rearrange("(a p) t -> p a t", p=128)[:])
                        maskT_t = [mt_b[:, k, :] for k in range(KT)]
                    q_self = proj_fm_ko(wq, x0cb, TOK, 'sbq', kv_pool, scale=0.125,
                                        tag_prefix="o_Q")
                with tc.tile_pool(name="acts_x0", bufs=1) as actp, \
                     tc.tile_pool(name="w_sWk", bufs=1) as wk_pool, \
                     tc.tile_pool(name="w_sWv", bufs=1) as wv_pool:
                    # interleave x0 k-tile and sWk part DMAs so the k-outer
                    # K projection starts after ~1.5MB of DMA
                    x0r = g['x0fm'].rearrange("(a p) t -> p a t", p=128)
                    wkr = g['sWk'].rearrange("(a p) d -> p a d", p=128)
                    x0_t, wk_tiles = [], []
                    for k in range(FT):
                        xt = actp.tile([128, S], BF, name=f"x0_{k}", tag=f"x0_{k}")
                        nc.sync.dma_start(xt[:], x0r[:, k, :])
                        x0_t.append(xt)
                        wt = wk_pool.tile([128, 1, D], BF, name=f"w_sWk_{k}", tag=f"w{k}")
                        nc.sync.dma_start(wt[:], wkr[:, k:k + 1, :])
                        wk_tiles.append(wt)
                    wk = lambda k, c0, c1: wk_tiles[k][:, 0, c0:c1]
                    wv = load_w_parts('sWv', wv_pool, parts=8)
                    k_self = proj_fm_ko(wk, x0_t, S, 'sbk', kv_pool, tag_prefix="o_K")
                    v_self = proj_tm_ko(wv, x0_t, free_bias['sbv'], kv_pool, tag_prefix="o_V")

                # phase 2: self attention + O-proj + LN1.  enc / sWo / x0c
                # DMAs are emitted before the attention body so they stream
                # during attention compute.
                with tc.tile_pool(name="acts_enc", bufs=1) as encp, \
                     tc.tile_pool(name="r1p", bufs=1) as r1_pool:
                    encr = g['encfm'].rearrange("(a p) t -> p a t", p=128)
                    enc_t = []
                    for k in range(FT):
                        et = encp.tile([128, S], BF, name=f"enc_{k}", tag=f"enc_{k}")
                        nc.sync.dma_start(et[:], encr[:, k, :])
                        enc_t.append(et)

                    with tc.tile_pool(name="w_sWo", bufs=1) as wo_pool_s, \
                         tc.tile_pool(name="x0c", bufs=1) as x0c_pool:
                        swo = load_w_parts('sWo', wo_pool_s, parts=2)
                        x0c_b = x0c_pool.tile([128, FT, TOK], F32, name="x0c_b", tag="x0c")
                        nc.sync.dma_start(x0c_b[:], g['x0chunk'].rearrange("(a p) t -> p a t", p=128)[:])
                        x0c_t = [x0c_b[:, k, :] for k in range(FT)]
                        with tc.tile_pool(name="at_s", bufs=1) as at_pool_s:
                            attn1 = attention(q_self, k_self, v_self, maskT_t, "s", at_pool_s)
                        r1 = o_proj_residual_ko(swo, "so", attn1, 'sbo', x0c_t, r1_pool)

                    with tc.tile_pool(name="w_cWv", bufs=1) as wcv_pool:
                        with tc.tile_pool(name="w_cWk", bufs=1) as wck_pool:
                            cwk = load_w_parts('cWk', wck_pool, parts=8)
                            cwv = load_w_parts('cWv', wcv_pool, parts=2)

                            def inter1():
                                return proj_fm('cWk', enc_t, S, 'cbk', kv_pool,
                                               tag_prefix="o_K", w_acc=cwk)
                            a1, a1b, k_cross = layer_norm(r1, 'ln1_g', 'ln1_b', F32,
                                                          a1pool, True, interleave=inter1)
                        v_cross = proj_tm_ko(cwv, enc_t, free_bias['cbv'], kv_pool,
                                             tag_prefix="o_V")

                # phase 3: cross attention + O-proj + LN2
                with tc.tile_pool(name="qc", bufs=1) as qc_pool, \
                     tc.tile_pool(name="maskc", bufs=1) as maskc_pool, \
                     tc.tile_pool(name="w_cWo", bufs=1) as wo_pool_c, \
                     tc.tile_pool(name="r2p", bufs=1) as r2_pool:
                    maskTc_t = None
                    if g['maskTc'] is not None:
                        mtc_b = maskc_pool.tile([128, KT, TOK], BF, name="mtc_b", tag="mtc")
                        nc.sync.dma_start(mtc_b[:], g['maskTc'].rearrange("(a p) t -> p a t", p=128)[:])
                        maskTc_t = [mtc_b[:, k, :] for k in range(KT)]
                    q_cross = proj_fm('cWq', a1b, TOK, 'cbq', qc_pool, scale=0.125)
                    cwo = load_w_parts('cWo', wo_pool_c, parts=2)
                    with tc.tile_pool(name="at_c", bufs=1) as at_pool_c:
                        attn2 = attention(q_cross, k_cross, v_cross, maskTc_t, "c", at_pool_c)
                        r2 = o_proj_residual_ko(cwo, "co", attn2, 'cbo', a1, r2_pool)
                    a2, a2b, _ = layer_norm(r2, 'ln2_g', 'ln2_b', F32, a2pool, True)

            # ================= phase 4: FFN + LN3 (chunked weights) =========
            with tc.tile_pool(name="r3p", bufs=1) as r3_pool:
                r3 = [r3_pool.tile([128, TOK], F32, name=f"r_ffn_{m}", tag=f"r{m}")
                      for m in range(FT)]
                with tc.tile_pool(name="w_f1", bufs=2) as wp1, \
                     tc.tile_pool(name="w_f2", bufs=2) as wp2, \
                     tc.tile_pool(name="hid", bufs=2) as hpool, \
                     tc.tile_pool(name="ps_f1", bufs=3, space=PSUM) as pp1, \
                     tc.tile_pool(name="ps_f2", bufs=3, space=PSUM) as pp2:
                    f1r = g['fW1'].rearrange("(a p) d -> p a d", p=128)
                    f2r = g['fW2'].rearrange("(a p) d -> p a d", p=128)
                    for ci in range(NCH):
                        w1c = wp1.tile([128, FT, CHT * 128], BF, tag="w1")
                        nc.sync.dma_start(w1c[:], f1r[:, :, ci * CHT * 128:(ci + 1) * CHT * 128])
                        w2c = wp2.tile([128, CHT, D], BF, tag="w2")
                        nc.sync.dma_start(w2c[:], f2r[:, ci * CHT:(ci + 1) * CHT, :])
                        hbuf = hpool.tile([128, CHT, TOK], BF, tag="h")
                        for mh in range(CHT):
                            ps = pp1.tile([128, TOK], F32, tag="ps")
                            for k in range(FT):
                                nc.tensor.matmul(ps[:], w1c[:, k, mh * 128:(mh + 1) * 128],
                                                 a2b[k][:], start=(k == 0), stop=(k == FT - 1))
                            nc.scalar.activation(hbuf[:, mh, :], ps[:],
                                                 mybir.ActivationFunctionType.Relu,
                                                 bias=fb1_sb[:, ci * CHT + mh:ci * CHT + mh + 1])
                        for m in range(FT):
                            ps = pp2.tile([128, TOK], F32, tag="ps")
                            for kh in range(CHT):
                                nc.tensor.matmul(ps[:], w2c[:, kh, m * 128:(m + 1) * 128],
                                                 hbuf[:, kh, :], start=(kh == 0), stop=(kh == CHT - 1))
                            if ci == 0:
                                nc.vector.scalar_tensor_tensor(r3[m][:], ps[:], bias_col('fb2', m),
                                                               a2[m][:], ADD, ADD)
                            else:
                                nc.vector.tensor_add(r3[m][:], r3[m][:], ps[:])
                y, _, _ = layer_norm(r3, 'ln3_g', 'ln3_b', BF, ypool, False)

            # Wout ring prefetch: queued behind the FFN weight DMAs, streams
            # during FFN/LN3 compute.
            for n in range(RING):
                emit_wo_load(n)

        # ================= phase 5: vocab projection ========================
        with tc.tile_pool(name="vout", bufs=4) as vos, \
             tc.tile_pool(name="vps", bufs=4, space=PSUM) as vps:
            for n in range(VN):
                wt = wo_tiles.pop(n)
                for m in range(2):
                    ps = vps.tile([128, VC], F32, tag="ps")
                    for k in range(FT):
                        nc.tensor.matmul(ps[:], y[k][:, m * 128:(m + 1) * 128],
                                         wt[:, k, :], start=(k == 0), stop=(k == FT - 1))
                    ot = vos.tile([128, VC], BF, tag="vo")
                    copy_out(ot[:], ps[:], n * 2 + m)
                    nc.gpsimd.dma_start(g['out'][m * 128:(m + 1) * 128, n * VC:(n + 1) * VC], ot[:])
                if n + RING < VN:
                    emit_wo_load(n + RING)


def host_prep(inputs):
    x0 = np.asarray(inputs['dec_input'], np.float32) + positional_encoding(S, D)[None]
    enc = np.asarray(inputs['enc_input'], np.float32)
    mask_self = np.asarray(inputs['masked_attention_mask'], np.float32)[0, 0]
    mask_cross = np.asarray(inputs['cross_attention_mask'], np.float32)[0, 0]
    self_adds = bool(np.any(mask_self != 0.0))
    cross_adds = bool(np.any(mask_cross != 0.0))
    li = L - 1
    Wl = {}
    for p in ['sWq', 'sWk', 'sWv', 'sWo', 'cWq', 'cWk', 'cWv', 'cWo', 'fW1', 'fW2']:
        Wl[p] = np.ascontiguousarray(np.asarray(inputs[p], np.float32)[li]).astype(BF16)
    bv = {}
    for p in ['sbq', 'sbk', 'sbv', 'sbo', 'cbq', 'cbk', 'cbv', 'cbo',
              'ln1_g', 'ln1_b', 'ln2_g', 'ln2_b', 'ln3_g', 'ln3_b', 'fb1', 'fb2']:
        bv[p] = np.asarray(inputs[p], np.float32)[li]
    Wout_bf = np.ascontiguousarray(np.asarray(inputs['Wout'], np.float32)).astype(BF16)
    bout = np.asarray(inputs['bout'], np.float32)
    ident = np.eye(128, dtype=BF16)

    def pp(v):  # [1024] -> [128, 8] partition-major
        return np.ascontiguousarray(v.reshape(-1, 128).T)

    bias_cols = []
    for name in BIAS_NAMES:
        src = {'sbq': bv['sbq'] * 0.125, 'cbq': bv['cbq'] * 0.125}.get(name, bv.get(name))
        bias_cols.append(pp(src))
    biases_pp = np.ascontiguousarray(np.concatenate(bias_cols, axis=1), np.float32)
    fb1_pp = np.ascontiguousarray(bv['fb1'].reshape(HT, 128).T, np.float32)

    in_maps = []
    for core in range(NC):
        b, c = core // 4, core % 4
        q0 = c * TOK
        x0c = np.ascontiguousarray(x0[b, q0:q0 + TOK].T)
        m = {
            'x0fm': np.ascontiguousarray(x0[b].T).astype(BF16),
            'encfm': np.ascontiguousarray(enc[b].T).astype(BF16),
            'x0chunk': np.ascontiguousarray(x0c, np.float32),
            'x0cb': x0c.astype(BF16),
            'biases': biases_pp, 'fb1': fb1_pp, 'ident': ident,
            'sbv_row': np.ascontiguousarray(bv['sbv'][None, :], np.float32),
            'cbv_row': np.ascontiguousarray(bv['cbv'][None, :], np.float32),
            'Wout': Wout_bf,
        }
        m.update(Wl)
        if self_adds:
            m['maskT'] = np.ascontiguousarray(mask_self[q0:q0 + TOK, :].T).astype(BF16)
        if cross_adds:
            m['maskTc'] = np.ascontiguousarray(mask_cross[q0:q0 + TOK, :].T).astype(BF16)
        in_maps.append(m)
    zero_free = not (np.any(bv['sbv']) or np.any(bv['cbv']))
    unit_ln = all(np.all(bv[f'ln{i}_g'] == 1.0) and not np.any(bv[f'ln{i}_b'])
                  for i in (1, 2, 3))
    return in_maps, self_adds, cross_adds, zero_free, unit_ln, bout


_CACHE = {}


def _get_program(self_adds, cross_adds, zero_free, unit_ln):
    key = (self_adds, cross_adds, zero_free, unit_ln)
    if key not in _CACHE:
        _CACHE[key] = build_program(self_adds, cross_adds, zero_free, unit_ln)
    return _CACHE[key]


def kernel(**inputs):
    in_maps, self_adds, cross_adds, zero_free, unit_ln, bout = host_prep(inputs)
    nc = _get_program(self_adds, cross_adds, zero_free, unit_ln)
    res = run_bass_kernel_spmd(nc, in_maps, core_ids=list(range(NC)))
    shards = [np.asarray(res.results[r]["out"], dtype=np.float32) for r in range(NC)]
    full = np.concatenate(shards, axis=0)           # [2048, V]
    if np.any(bout):
        full = full + bout[None, :]
    return np.ascontiguousarray(full.reshape(B, S, V), np.float32)
